# revision 1
# baseline (speedup 1.0000x reference)
"""Multi-head attention (RoPE + mask + softmax) Trainium2 Bass kernel.

Sharding: 8 cores = 2 batches x 4 head-groups. Core c handles batch c//4,
local heads 4*(c%4) .. +4 (tensor-parallel on heads; Wq/Wk/Wv column-sharded,
Wo row-sharded; per-core partial outputs summed on host).

All DRAM inputs are host-pre-tiled so every DMA is partition-contiguous
(~128 descriptors). Per-core pipeline (S=2048, 4 heads of dim 64):
  qhT/khT = (Wq_perm)^T @ q^T   [2x128, 2048] f32r   (PE, K=1024 accum)
  RoPE fused into psum eviction: t = psum*cos, u = psum*sin_signed (DVE),
    swap 32-row blocks of u via SBUF->SBUF DMA (gpsimd queue), add (DVE)
  vh = v @ Wv  [2048, 4*65] bf16 with ones column per head (PE + strided ACT evict)
  per (q-block 1024, head-pair, k-chunk, head): scoresT[k,q] (PE, K=64),
    exp(x/8) (ACT psum->bf16), mask-mul (DVE bf16),
    attn@V accumulate [65, 1024] (PE bf16; row 64 = softmax denominator)
  denominators per (qb, head-pair): reciprocal_approx_accurate on a [128, 16]
    reshape (DRAM bounce), PE K=1 ones-broadcast, DVE normalize -> outT f32r
  out_part = outT^T @ Wo  (PE, 4x K=64 accum) -> [2048, 1024] f32
"""
import sys
sys.path.insert(0, '/opt/trn_rl_repo')
import math
import numpy as np
import ml_dtypes

import concourse.bass as bass
import concourse.mybir as mybir
import concourse.tile as tile
from concourse import bacc
from concourse.bass_utils import run_bass_kernel_spmd

F32 = mybir.dt.float32
F32R = mybir.dt.float32r
BF16 = mybir.dt.bfloat16

S = 2048
DIM = 1024
HEAD_DIM = 64
N_CORES = 8
KC = DIM // 128          # 8 contraction chunks for projections
MT = S // 128            # 16 k-chunks in attention
QB = 1024                # q-block width
NQB = S // QB            # 2
ROPE_THETA = 10000.0

_BUILT = None


def build_bass():
    nc = bacc.Bacc("TRN2", target_bir_lowering=False, debug=False)

    qT = nc.dram_tensor("qT", [4, 128, KC, 512], F32R, kind="ExternalInput").ap()
    kT = nc.dram_tensor("kT", [4, 128, KC, 512], F32R, kind="ExternalInput").ap()
    vT = nc.dram_tensor("vT", [MT, 128, KC, 128], F32R, kind="ExternalInput").ap()
    wq = nc.dram_tensor("wq", [128, KC, 256], F32R, kind="ExternalInput").ap()
    wk = nc.dram_tensor("wk", [128, KC, 256], F32R, kind="ExternalInput").ap()
    wv = nc.dram_tensor("wv", [128, KC, 256], F32R, kind="ExternalInput").ap()
    wo = nc.dram_tensor("wo", [64, 4, DIM], F32R, kind="ExternalInput").ap()
    cosT = nc.dram_tensor("cosT", [128, S], F32, kind="ExternalInput").ap()
    sinT = nc.dram_tensor("sinT", [128, S], F32, kind="ExternalInput").ap()
    maskT = nc.dram_tensor("maskT", [128, MT, S], BF16, kind="ExternalInput").ap()
    ones64 = nc.dram_tensor("ones64", [1, 64], F32R, kind="ExternalInput").ap()
    out_part = nc.dram_tensor("out_part", [S, DIM], F32, kind="ExternalOutput").ap()

    with tile.TileContext(nc) as tc:
        with tc.tile_pool(name="persist", bufs=1) as persist, \
             tc.tile_pool(name="dram", bufs=1, space="DRAM") as dram, \
             tc.tile_pool(name="ps", bufs=4, space="PSUM") as ps:

            qhT = persist.tile([128, 2, S], F32R)     # [chunk-part, chunk, s]
            khT = persist.tile([128, 2, S], F32R)
            vh = persist.tile([128, MT, 4 * 65], BF16)
            outT = persist.tile([64, 4, S], F32R)
            wo_sb = persist.tile([64, 4, DIM], F32R)
            ones_sb = persist.tile([1, 64], F32R)
            dscr = dram.tile([8, QB], F32)
            dscr2 = dram.tile([8, QB], F32R)

            nc.sync.dma_start(out=wo_sb, in_=wo)
            nc.sync.dma_start(out=ones_sb, in_=ones64)
            # ones column for the denominator rows of vh
            nc.vector.memset(
                vh.rearrange("p m (h x) -> p m h x", x=65)[:, :, :, 64:65], 1.0)

            # ---------------- Phase 1+2: projections + RoPE ----------------
            with tc.tile_pool(name="proj", bufs=1) as projp, \
                 tc.tile_pool(name="xts", bufs=2) as xts, \
                 tc.tile_pool(name="rope", bufs=2) as rope:
                wq_sb = projp.tile([128, KC, 256], F32R)
                wk_sb = projp.tile([128, KC, 256], F32R)
                wv_sb = projp.tile([128, KC, 256], F32R)
                cos_sb = projp.tile([128, S], F32)
                sin_sb = projp.tile([128, S], F32)
                nc.sync.dma_start(out=wq_sb, in_=wq)
                nc.sync.dma_start(out=wk_sb, in_=wk)
                nc.sync.dma_start(out=wv_sb, in_=wv)
                nc.sync.dma_start(out=cos_sb, in_=cosT)
                nc.sync.dma_start(out=sin_sb, in_=sinT)

                # q/k projections with fused RoPE eviction
                for xdram, w_sb, dstT in ((qT, wq_sb, qhT), (kT, wk_sb, khT)):
                    for sblk in range(4):
                        x_sb = xts.tile([128, KC, 512], F32R, tag="xts")
                        nc.sync.dma_start(out=x_sb, in_=xdram[sblk])
                        ss = slice(sblk * 512, (sblk + 1) * 512)
                        for m in range(2):
                            psum = ps.tile([128, QB], F32, tag="ps")
                            for kc in range(KC):
                                nc.tensor.matmul(
                                    psum[:, 0:512],
                                    lhsT=w_sb[:, kc, m * 128:(m + 1) * 128],
                                    rhs=x_sb[:, kc, :],
                                    start=(kc == 0), stop=(kc == KC - 1))
                            t = rope.tile([128, 512], F32, tag="t")
                            u = rope.tile([128, 512], F32, tag="u")
                            nc.vector.tensor_mul(t, psum[:, 0:512], cos_sb[:, ss])
                            nc.vector.tensor_mul(u, psum[:, 0:512], sin_sb[:, ss])
                            us = rope.tile([128, 512], F32, tag="us")
                            for blk in range(4):
                                a, b2 = blk * 32, (blk ^ 1) * 32
                                nc.gpsimd.dma_start(out=us[a:a + 32, :],
                                                    in_=u[b2:b2 + 32, :])
                            nc.vector.tensor_add(dstT[:, m, ss], t, us)

                # v projection with strided bf16 eviction (+ ones cols preset)
                for sc in range(MT):
                    v_sb = xts.tile([128, KC, 128], F32R, tag="xts")
                    nc.sync.dma_start(out=v_sb, in_=vT[sc])
                    psum = ps.tile([128, QB], F32, tag="ps")
                    for kc in range(KC):
                        nc.tensor.matmul(
                            psum[:, 0:256], lhsT=v_sb[:, kc, :], rhs=wv_sb[:, kc, :],
                            start=(kc == 0), stop=(kc == KC - 1))
                    nc.scalar.copy(
                        vh[:, sc, :].rearrange("p (h x) -> p h x", x=65)[:, :, 0:64],
                        psum[:, 0:256].rearrange("p (h x) -> p h x", x=64))

            # ---------------- Phase 3: attention ----------------
            with tc.tile_pool(name="mask", bufs=1) as maskp, \
                 tc.tile_pool(name="attn", bufs=3) as attnp, \
                 tc.tile_pool(name="dn", bufs=2) as dnp:
                mk = maskp.tile([128, MT, S], BF16, tag="mask")
                for mq in range(4):
                    nc.sync.dma_start(out=mk[:, mq * 4:(mq + 1) * 4, :],
                                      in_=maskT[:, mq * 4:(mq + 1) * 4, :])
                stg = dnp.tile([128, QB], F32, tag="stg")
                for qb in range(NQB):
                    qs = slice(qb * QB, (qb + 1) * QB)
                    for hp in range(2):
                        avp = [ps.tile([128, QB], F32, tag="ps", name=f"avp{_i}")
                               for _i in range(2)]
                        for m in range(MT):
                            for h2 in range(2):
                                hb = slice(h2 * 64, (h2 + 1) * 64)
                                sps = ps.tile([128, QB], F32, tag="ps")
                                for q2 in range(2):
                                    q5 = slice(q2 * 512, (q2 + 1) * 512)
                                    nc.tensor.matmul(
                                        sps[:, q5],
                                        lhsT=khT[hb, hp, m * 128:(m + 1) * 128],
                                        rhs=qhT[hb, hp, qb * QB + q2 * 512:
                                                qb * QB + (q2 + 1) * 512],
                                        start=True, stop=True)
                                at = attnp.tile([128, QB], BF16, tag="at")
                                nc.scalar.activation(
                                    at, sps, mybir.ActivationFunctionType.Exp,
                                    scale=1.0 / math.sqrt(HEAD_DIM))
                                atm = attnp.tile([128, QB], BF16, tag="atm")
                                nc.vector.tensor_mul(atm, at, mk[:, m, qs])
                                h = 2 * hp + h2
                                for q2 in range(2):
                                    q5 = slice(q2 * 512, (q2 + 1) * 512)
                                    nc.tensor.matmul(
                                        avp[h2][0:65, q5],
                                        lhsT=vh[:, m, h * 65:(h + 1) * 65],
                                        rhs=atm[:, q5],
                                        start=(m == 0), stop=(m == MT - 1))
                        # evict + normalize this (qb, head-pair) right away
                        for h2 in range(2):
                            h = 2 * hp + h2
                            unit = qb * 4 + hp * 2 + h2
                            nc.vector.tensor_copy(outT[0:64, h, qs], avp[h2][0:64, :])
                            nc.scalar.copy(stg[64:65, :], avp[h2][64:65, :])
                            nc.sync.dma_start(out=dscr[unit, :], in_=stg[64:65, :])
                        u0 = qb * 4 + hp * 2
                        rin = dnp.tile([128, 2, 8], F32, tag="rin")
                        nc.sync.dma_start(
                            out=rin,
                            in_=dscr[u0:u0 + 2].rearrange("u (p f) -> p u f", p=128))
                        r32 = dnp.tile([128, 2, 8], F32, tag="r32")
                        scr = dnp.tile([128, 2, 8], F32, tag="scr")
                        nc.vector.reciprocal_approx_accurate(r32, rin, scr)
                        rr = dnp.tile([128, 2, 8], F32R, tag="rr")
                        nc.vector.tensor_copy(rr, r32)
                        nc.sync.dma_start(
                            out=dscr2[u0:u0 + 2].rearrange("u (p f) -> p u f", p=128),
                            in_=rr)
                        for h2 in range(2):
                            h = 2 * hp + h2
                            unit = u0 + h2
                            rdn = dnp.tile([1, QB], F32R, tag="rdn")
                            nc.sync.dma_start(out=rdn, in_=dscr2[unit:unit + 1, :])
                            pbc = ps.tile([128, QB], F32, tag="ps")
                            for q2 in range(2):
                                q5 = slice(q2 * 512, (q2 + 1) * 512)
                                nc.tensor.matmul(pbc[0:64, q5], lhsT=ones_sb,
                                                 rhs=rdn[:, q5], start=True, stop=True)
                            nc.vector.tensor_mul(outT[0:64, h, qs],
                                                 outT[0:64, h, qs], pbc[0:64, :])

            # ---------------- Phase 5: output projection ----------------
            with tc.tile_pool(name="outp", bufs=3) as outp:
                for sc in range(MT):
                    wps = ps.tile([128, QB], F32, tag="ps")
                    for nb in range(2):
                        n5 = slice(nb * 512, (nb + 1) * 512)
                        for h in range(4):
                            nc.tensor.matmul(
                                wps[:, n5],
                                lhsT=outT[0:64, h, sc * 128:(sc + 1) * 128],
                                rhs=wo_sb[0:64, h, n5],
                                start=(h == 0), stop=(h == 3))
                    co = outp.tile([128, DIM], F32, tag="co")
                    nc.scalar.copy(co, wps)
                    nc.sync.dma_start(out=out_part[sc * 128:(sc + 1) * 128, :], in_=co)

    nc.compile()
    return nc


def _rope_perm_cols():
    """Column permutation of the 256-wide W slice for one core's 4 heads.

    Chunk c (0,1) holds local heads 2c, 2c+1 as rows
    [hA_even(32) | hA_odd(32) | hB_even(32) | hB_odd(32)].
    """
    cols = []
    for c in range(2):
        for j2 in range(2):          # which head within the chunk
            head = 2 * c + j2
            for blk in range(2):     # 0: even dims, 1: odd dims
                for i in range(32):
                    cols.append(head * 64 + 2 * i + blk)
    return np.array(cols)


def _cos_sin_tables():
    inv_freq = 1.0 / (ROPE_THETA ** (np.arange(0, HEAD_DIM, 2, dtype=np.float64)
                                     / HEAD_DIM))          # [32]
    ang = np.arange(S, dtype=np.float64)[None, :] * inv_freq[:, None]  # [32, S]
    cos32 = np.cos(ang).astype(np.float32)
    sin32 = np.sin(ang).astype(np.float32)
    cosT = np.tile(cos32, (4, 1))                           # [128, S]
    # sign: +sin at even-dim rows (blocks 0, 2), -sin at odd-dim rows (1, 3)
    sinT = np.concatenate([sin32, -sin32, sin32, -sin32], axis=0)
    return np.ascontiguousarray(cosT), np.ascontiguousarray(sinT)


def _tile_xT(xT):
    # [1024, 2048] -> [4 sblk, 128 part, 8 kc, 512]
    return np.ascontiguousarray(
        xT.reshape(KC, 128, 4, 512).transpose(2, 1, 0, 3))


def _tile_vT(vT):
    # [1024, 2048] -> [16 sc, 128 part, 8 kc, 128]
    return np.ascontiguousarray(
        vT.reshape(KC, 128, MT, 128).transpose(2, 1, 0, 3))


def _tile_w(w):
    # [1024, 256] -> [128, 8, 256]
    return np.ascontiguousarray(w.reshape(KC, 128, 256).transpose(1, 0, 2))


def _tile_mask(maskT_bf16):
    # [2048, 2048] -> [128, 16 m, 2048]
    return np.ascontiguousarray(
        maskT_bf16.reshape(MT, 128, S).transpose(1, 0, 2))


def kernel(q, k, v, mask, Wq, Wk, Wv, Wo, bo):
    global _BUILT
    if _BUILT is None:
        _BUILT = build_bass()
    nc = _BUILT

    q = np.asarray(q, np.float32)
    k = np.asarray(k, np.float32)
    v = np.asarray(v, np.float32)
    Wq = np.asarray(Wq, np.float32)
    Wk = np.asarray(Wk, np.float32)
    Wv = np.asarray(Wv, np.float32)
    Wo = np.asarray(Wo, np.float32)
    bo = np.asarray(bo, np.float32)
    mask = np.asarray(mask)

    cosT, sinT = _cos_sin_tables()
    ones64 = np.ones((1, 64), np.float32)
    perm = _rope_perm_cols()
    qTb = [_tile_xT(q[b].T) for b in range(2)]
    kTb = [_tile_xT(k[b].T) for b in range(2)]
    vTb = [_tile_vT(v[b].T) for b in range(2)]
    maskTb = [_tile_mask(mask[b, 0].T.astype(ml_dtypes.bfloat16)) for b in range(2)]

    in_maps = []
    for c in range(N_CORES):
        b = c // 4
        head_base = (c % 4) * 4
        cols = slice(head_base * 64, head_base * 64 + 256)
        in_maps.append({
            "qT": qTb[b], "kT": kTb[b], "vT": vTb[b],
            "wq": _tile_w(Wq[:, cols][:, perm]),
            "wk": _tile_w(Wk[:, cols][:, perm]),
            "wv": _tile_w(Wv[:, cols]),
            "wo": np.ascontiguousarray(
                Wo[cols, :].reshape(4, 64, DIM).transpose(1, 0, 2)),
            "cosT": cosT, "sinT": sinT,
            "maskT": maskTb[b], "ones64": ones64,
        })

    kernel._last_in_maps = in_maps
    res = run_bass_kernel_spmd(nc, in_maps, core_ids=list(range(N_CORES)))
    out = np.zeros((2, S, DIM), np.float32)
    for c in range(N_CORES):
        out[c // 4] += res.results[c]["out_part"]
    out += bo[None, None, :]
    return out



# revision 11
# speedup vs baseline: 1.6216x; 1.6216x over previous
"""Multi-head attention (RoPE + mask + softmax) Trainium2 Bass kernel, v2.

Sharding: 8 cores = 2 batches x 4 head-groups. Core c handles batch c//4,
local heads 4*(c%4) .. +4 (tensor-parallel on heads; Wq/Wk/Wv column-sharded,
Wo row-sharded; per-core partial outputs summed on host).

v2 design (vs f32r v1): all-bf16 datapath, scores row-tiled by head pair,
mask folded into PE via identity-matmul accumulation of -16384*(1-mask)
(fp8e5), exp at N=1024 per (hp, m) chunk with PSUM double-buffering.
Attention is ScalarE(exp)-bound by design; v-proj/out-proj/normalize are
interleaved into PE/DVE slack.

Per-core layout: local head h = 2*hp + h2; qhT/khT [128, hp, s] with
h2=0 on partitions 0-63, h2=1 on 64-127 (each head pre-permuted
[even32|odd32] for interleaved RoPE).
"""
import sys
sys.path.insert(0, '/opt/trn_rl_repo')
import math
import numpy as np
import ml_dtypes

import concourse.bass as bass
import concourse.mybir as mybir
import concourse.tile as tile
from concourse import bacc
from concourse.bass_utils import run_bass_kernel_spmd

F32 = mybir.dt.float32
BF16 = mybir.dt.bfloat16
F8E5 = mybir.dt.float8e5

S = 2048
DIM = 1024
HEAD_DIM = 64
N_CORES = 8
KC = DIM // 128          # 8 contraction chunks for projections
MT = S // 128            # 16 k-chunks in attention
QB = 512                 # q-block width
NQB = S // QB            # 4
ROPE_THETA = 10000.0
MASK_NEG = -16384.0      # exactly representable in fp8e5; (s-16384)/8 -> exp=0

_BUILT = None
DEBUG_DUMPS = False


def build_bass():
    nc = bacc.Bacc("TRN2", target_bir_lowering=False, debug=False)

    qT = nc.dram_tensor("qT", [4, 128, KC, 512], BF16, kind="ExternalInput").ap()
    kT = nc.dram_tensor("kT", [4, 128, KC, 512], BF16, kind="ExternalInput").ap()
    vT = nc.dram_tensor("vT", [MT, 128, KC, 128], BF16, kind="ExternalInput").ap()
    wq = nc.dram_tensor("wq", [128, KC, 256], BF16, kind="ExternalInput").ap()
    wk = nc.dram_tensor("wk", [128, KC, 256], BF16, kind="ExternalInput").ap()
    wv = nc.dram_tensor("wv", [128, KC, 256], BF16, kind="ExternalInput").ap()
    wo = nc.dram_tensor("wo", [64, 4, DIM], BF16, kind="ExternalInput").ap()
    cosT = nc.dram_tensor("cosT", [128, S], BF16, kind="ExternalInput").ap()
    sinT = nc.dram_tensor("sinT", [128, S], BF16, kind="ExternalInput").ap()
    mkneg = nc.dram_tensor("mkneg", [128, MT, S], F8E5, kind="ExternalInput").ap()
    ident = nc.dram_tensor("ident", [128, 128], F8E5, kind="ExternalInput").ap()
    ones64 = nc.dram_tensor("ones64", [1, 64], BF16, kind="ExternalInput").ap()
    out_part = nc.dram_tensor("out_part", [S, DIM], BF16, kind="ExternalOutput").ap()
    if DEBUG_DUMPS:
        dbg_qhT = nc.dram_tensor("dbg_qhT", [128, 2, S], BF16,
                                 kind="ExternalOutput").ap()
        dbg_khT = nc.dram_tensor("dbg_khT", [128, 2, S], BF16,
                                 kind="ExternalOutput").ap()
        dbg_vh = nc.dram_tensor("dbg_vh", [128, MT, 4 * 65], BF16,
                                kind="ExternalOutput").ap()
        dbg_at = nc.dram_tensor("dbg_at", [8, 128, 1024], BF16,
                                kind="ExternalOutput").ap()
        dbg_outT2 = nc.dram_tensor("dbg_outT2", [64, 4, S], BF16,
                                   kind="ExternalOutput").ap()
        dbg_outT2n = nc.dram_tensor("dbg_outT2n", [64, 4, S], BF16,
                                    kind="ExternalOutput").ap()

    with tile.TileContext(nc) as tc:
        with tc.tile_pool(name="persist", bufs=1) as persist, \
             tc.tile_pool(name="dram", bufs=1, space="DRAM") as dram, \
             tc.tile_pool(name="ps_sc", bufs=2, space="PSUM") as ps_sc, \
             tc.tile_pool(name="ps_avp", bufs=1, space="PSUM") as ps_avp, \
             tc.tile_pool(name="ps_misc", bufs=2, space="PSUM") as ps_misc:

            qhT = persist.tile([128, 2, S], BF16)     # [dim-part, hp, s]
            khT = persist.tile([128, 2, S], BF16)
            vh = persist.tile([128, MT, 4 * 65], BF16)
            outT2 = persist.tile([64, 4, S], BF16)
            wo_sb = persist.tile([64, 4, DIM], BF16)
            ones_sb = persist.tile([1, 64], BF16)
            id_sb = persist.tile([128, 128], F8E5)
            cos_sb = persist.tile([128, S], BF16)
            sin_sb = persist.tile([128, S], BF16)
            mkneg_sb = persist.tile([128, MT, S], F8E5)
            mkmult_sb = persist.tile([128, MT, S], BF16)
            dscr = dram.tile([2 * NQB, 2 * QB], F32)
            dscr2 = dram.tile([2 * NQB, 2 * QB], BF16)

            nc.sync.dma_start(out=wo_sb, in_=wo)
            nc.sync.dma_start(out=ones_sb, in_=ones64)
            nc.sync.dma_start(out=id_sb, in_=ident)
            nc.sync.dma_start(out=cos_sb, in_=cosT)
            nc.sync.dma_start(out=sin_sb, in_=sinT)
            # ones column for the denominator rows of vh
            nc.vector.memset(
                vh.rearrange("p m (h x) -> p m h x", x=65)[:, :, :, 64:65], 1.0)

            # ---------------- Phase 1: q/k projections + RoPE ----------------
            with tc.tile_pool(name="proj", bufs=1) as projp, \
                 tc.tile_pool(name="xts", bufs=3) as xts, \
                 tc.tile_pool(name="rope", bufs=2) as rope:
                wq_sb = projp.tile([128, KC, 256], BF16)
                wk_sb = projp.tile([128, KC, 256], BF16)
                wv_sb = projp.tile([128, KC, 256], BF16)
                nc.sync.dma_start(out=wq_sb, in_=wq)
                nc.sync.dma_start(out=wk_sb, in_=wk)
                nc.sync.dma_start(out=wv_sb, in_=wv)

                for xdram, w_sb, dstT in ((qT, wq_sb, qhT), (kT, wk_sb, khT)):
                    for sblk in range(4):
                        x_sb = xts.tile([128, KC, 512], BF16, tag="xts")
                        nc.sync.dma_start(out=x_sb, in_=xdram[sblk])
                        ss = slice(sblk * 512, (sblk + 1) * 512)
                        for hp in range(2):
                            psum = ps_sc.tile([128, 1024], F32, tag="sc")
                            for kc in range(KC):
                                nc.tensor.matmul(
                                    psum[:, 0:512],
                                    lhsT=w_sb[:, kc, hp * 128:(hp + 1) * 128],
                                    rhs=x_sb[:, kc, :],
                                    start=(kc == 0), stop=(kc == KC - 1))
                            qraw = rope.tile([128, 512], BF16, tag="qraw")
                            nc.vector.tensor_copy(qraw, psum[:, 0:512])
                            t = rope.tile([128, 512], BF16, tag="t")
                            u = rope.tile([128, 512], BF16, tag="u")
                            nc.vector.tensor_mul(t, qraw, cos_sb[:, ss])
                            nc.vector.tensor_mul(u, qraw, sin_sb[:, ss])
                            us = rope.tile([128, 512], BF16, tag="us")
                            for blk in range(4):
                                a, b2 = blk * 32, (blk ^ 1) * 32
                                nc.gpsimd.dma_start(out=us[a:a + 32, :],
                                                    in_=u[b2:b2 + 32, :])
                            nc.vector.tensor_add(dstT[:, hp, ss], t, us)

                # ---------------- Phase 2: v projection ----------------
                # (mkneg chunks interleaved with v tiles so the mask stream
                # trails v in the DMA queue but stays ahead of attention use)
                for sc in range(4):
                    nc.sync.dma_start(out=mkneg_sb[:, sc * 2:sc * 2 + 2, :],
                                      in_=mkneg[:, sc * 2:sc * 2 + 2, :])
                for sc in range(MT):
                    v_sb = xts.tile([128, KC, 128], BF16, tag="xts")
                    nc.sync.dma_start(out=v_sb, in_=vT[sc])
                    if sc * 2 + 8 + 1 < MT:
                        m0 = 8 + sc * 2
                        nc.sync.dma_start(out=mkneg_sb[:, m0:m0 + 2, :],
                                          in_=mkneg[:, m0:m0 + 2, :])
                    psum = ps_misc.tile([128, 512], F32, tag="misc")
                    for kc in range(KC):
                        nc.tensor.matmul(
                            psum[:, 0:256], lhsT=v_sb[:, kc, :], rhs=wv_sb[:, kc, :],
                            start=(kc == 0), stop=(kc == KC - 1))
                    nc.scalar.copy(
                        vh[:, sc, :].rearrange("p (h x) -> p h x", x=65)[:, :, 0:64],
                        psum[:, 0:256].rearrange("p (h x) -> p h x", x=64))

            # ---------------- Phase 3: attention ----------------
            with tc.tile_pool(name="at", bufs=3) as atp, \
                 tc.tile_pool(name="atm", bufs=3) as atmp, \
                 tc.tile_pool(name="stg", bufs=2) as stgp, \
                 tc.tile_pool(name="dn", bufs=2) as dnp, \
                 tc.tile_pool(name="co", bufs=3) as cop:

                def outproj_chunk(qbo, j):
                    # out-projection chunk j (of 8) for q-block qbo
                    sci, nb = qbo * 4 + j // 2, j % 2
                    n5 = slice(nb * 512, (nb + 1) * 512)
                    wps = ps_misc.tile([128, 512], F32, tag="misc")
                    for h in range(4):
                        nc.tensor.matmul(
                            wps,
                            lhsT=outT2[0:64, h, sci * 128:(sci + 1) * 128],
                            rhs=wo_sb[0:64, h, n5],
                            start=(h == 0), stop=(h == 3))
                    co = cop.tile([128, 512], BF16, tag="co")
                    nc.vector.tensor_copy(co, wps)
                    nc.sync.dma_start(
                        out=out_part[sci * 128:(sci + 1) * 128, n5], in_=co)

                for qb in range(NQB):
                    qs = slice(qb * QB, (qb + 1) * QB)
                    for hp in range(2):
                        avp = ps_avp.tile([128, 1024], F32, tag="avp")
                        for m in range(MT):
                            it = hp * MT + m
                            sc = ps_sc.tile([128, 1024], F32, tag="sc")
                            # scores: row-tiled concurrent head pair (K=64)
                            nc.tensor.matmul(
                                sc[:, 0:512],
                                lhsT=khT[0:64, hp, m * 128:(m + 1) * 128],
                                rhs=qhT[0:64, hp, qs],
                                start=True, stop=False, tile_position=(0, 0))
                            nc.tensor.matmul(
                                sc[:, 512:1024],
                                lhsT=khT[64:128, hp, m * 128:(m + 1) * 128],
                                rhs=qhT[64:128, hp, qs],
                                start=True, stop=(qb > 0), tile_position=(64, 0))
                            # mask h2=0 via PE accumulation (and h2=1 on qb==0,
                            # while DVE decodes the multiplicative mask)
                            nc.tensor.matmul(
                                sc[:, 0:512], lhsT=id_sb,
                                rhs=mkneg_sb[:, m, qs],
                                start=False, stop=True)
                            if qb == 0:
                                nc.tensor.matmul(
                                    sc[:, 512:1024], lhsT=id_sb,
                                    rhs=mkneg_sb[:, m, qs],
                                    start=False, stop=True)
                            at = atp.tile([128, 1024], BF16, tag="at")
                            nc.scalar.activation(
                                at, sc, mybir.ActivationFunctionType.Exp,
                                scale=1.0 / math.sqrt(HEAD_DIM))
                            if DEBUG_DUMPS and qb == 0 and hp == 0 and m < 8:
                                nc.sync.dma_start(out=dbg_at[m], in_=at)
                            if qb == 0 and hp == 0:
                                # decode multiplicative mask: 1 + mkneg/16384
                                # (mkneg is -16384 at masked positions, 0 else)
                                nc.vector.tensor_scalar(
                                    mkmult_sb[:, m, :], mkneg_sb[:, m, :],
                                    1.0 / 16384.0, 1.0,
                                    op0=mybir.AluOpType.mult,
                                    op1=mybir.AluOpType.add)
                            if qb > 0:
                                atm1 = atmp.tile([128, 512], BF16, tag="atm")
                                nc.vector.tensor_mul(
                                    atm1, at[:, 512:1024], mkmult_sb[:, m, qs])
                                rhs1 = atm1
                            else:
                                rhs1 = at[:, 512:1024]
                            # attn @ V (K=128, M=65 with denominator column)
                            h0, h1 = 2 * hp, 2 * hp + 1
                            nc.tensor.matmul(
                                avp[0:65, 0:512],
                                lhsT=vh[:, m, h0 * 65:(h0 + 1) * 65],
                                rhs=at[:, 0:512],
                                start=(m == 0), stop=(m == MT - 1))
                            nc.tensor.matmul(
                                avp[0:65, 512:1024],
                                lhsT=vh[:, m, h1 * 65:(h1 + 1) * 65],
                                rhs=rhs1,
                                start=(m == 0), stop=(m == MT - 1))
                            # interleave prev q-block's out-projection chunks
                            if qb > 0 and it >= 8 and (it - 8) % 3 == 0 \
                                    and (it - 8) // 3 < 8:
                                outproj_chunk(qb - 1, (it - 8) // 3)
                        # evict this (qb, hp): data rows + denominator row
                        for h2 in range(2):
                            nc.vector.tensor_copy(
                                outT2[0:64, 2 * hp + h2, qs],
                                avp[0:64, h2 * 512:(h2 + 1) * 512])
                        if DEBUG_DUMPS:
                            for h2 in range(2):
                                nc.sync.dma_start(
                                    out=dbg_outT2[:, 2 * hp + h2, qs],
                                    in_=outT2[0:64, 2 * hp + h2, qs])
                        stg = stgp.tile([128, 1024], F32, tag="stg")
                        nc.vector.tensor_copy(stg[64:65, :], avp[64:65, :])
                        nc.sync.dma_start(out=dscr[qb * 2 + hp, :],
                                          in_=stg[64:65, :])
                    # denominators for this q-block: bounce, reciprocal, norm
                    rin = dnp.tile([128, 2, 8], F32, tag="rin")
                    nc.sync.dma_start(
                        out=rin,
                        in_=dscr[qb * 2:qb * 2 + 2].rearrange(
                            "u (p f) -> p u f", p=128))
                    r32 = dnp.tile([128, 2, 8], F32, tag="r32")
                    scr = dnp.tile([128, 2, 8], F32, tag="scr")
                    nc.vector.reciprocal_approx_accurate(r32, rin, scr)
                    rr = dnp.tile([128, 2, 8], BF16, tag="rr")
                    nc.vector.tensor_copy(rr, r32)
                    nc.sync.dma_start(
                        out=dscr2[qb * 2:qb * 2 + 2].rearrange(
                            "u (p f) -> p u f", p=128),
                        in_=rr)
                    for hp in range(2):
                        for h2 in range(2):
                            h = 2 * hp + h2
                            rdn = dnp.tile([1, 512], BF16, tag="rdn")
                            nc.sync.dma_start(
                                out=rdn,
                                in_=dscr2[qb * 2 + hp:qb * 2 + hp + 1,
                                          h2 * 512:(h2 + 1) * 512])
                            pbc = ps_misc.tile([128, 512], F32, tag="misc")
                            nc.tensor.matmul(pbc[0:64, :], lhsT=ones_sb,
                                             rhs=rdn, start=True, stop=True)
                            pbcb = dnp.tile([64, 512], BF16, tag="pbcb")
                            nc.vector.tensor_copy(pbcb, pbc[0:64, :])
                            nc.vector.tensor_mul(outT2[0:64, h, qs],
                                                 outT2[0:64, h, qs], pbcb)
                # tail: out-projection for the last q-block
                for j in range(8):
                    outproj_chunk(NQB - 1, j)
                if DEBUG_DUMPS:
                    nc.sync.dma_start(out=dbg_qhT, in_=qhT)
                    nc.sync.dma_start(out=dbg_khT, in_=khT)
                    nc.sync.dma_start(out=dbg_vh, in_=vh)
                    nc.sync.dma_start(out=dbg_outT2n, in_=outT2)

    nc.compile()
    return nc


def _rope_perm_cols():
    """Column permutation of the 256-wide W slice for one core's 4 heads.

    Chunk hp (0,1) holds local heads 2hp, 2hp+1 as partition halves
    [hA_even(32) | hA_odd(32) | hB_even(32) | hB_odd(32)].
    """
    cols = []
    for c in range(2):
        for j2 in range(2):          # which head within the chunk
            head = 2 * c + j2
            for blk in range(2):     # 0: even dims, 1: odd dims
                for i in range(32):
                    cols.append(head * 64 + 2 * i + blk)
    return np.array(cols)


def _cos_sin_tables():
    inv_freq = 1.0 / (ROPE_THETA ** (np.arange(0, HEAD_DIM, 2, dtype=np.float64)
                                     / HEAD_DIM))          # [32]
    ang = np.arange(S, dtype=np.float64)[None, :] * inv_freq[:, None]  # [32, S]
    cos32 = np.cos(ang)
    sin32 = np.sin(ang)
    cosT = np.tile(cos32, (4, 1)).astype(ml_dtypes.bfloat16)  # [128, S]
    # sign: +sin at even-dim rows (blocks 0, 2), -sin at odd-dim rows (1, 3)
    sinT = np.concatenate([sin32, -sin32, sin32, -sin32],
                          axis=0).astype(ml_dtypes.bfloat16)
    return np.ascontiguousarray(cosT), np.ascontiguousarray(sinT)


def _tile_xT(xT):
    # [1024, 2048] -> [4 sblk, 128 part, 8 kc, 512]
    return np.ascontiguousarray(
        xT.reshape(KC, 128, 4, 512).transpose(2, 1, 0, 3))


def _tile_vT(vT):
    # [1024, 2048] -> [16 sc, 128 part, 8 kc, 128]
    return np.ascontiguousarray(
        vT.reshape(KC, 128, MT, 128).transpose(2, 1, 0, 3))


def _tile_w(w):
    # [1024, 256] -> [128, 8, 256]
    return np.ascontiguousarray(w.reshape(KC, 128, 256).transpose(1, 0, 2))


def _tile_mask(maskT):
    # [2048 k, 2048 q] -> [128, 16 m, 2048]
    return np.ascontiguousarray(
        maskT.reshape(MT, 128, S).transpose(1, 0, 2))


def kernel(q, k, v, mask, Wq, Wk, Wv, Wo, bo):
    global _BUILT
    if _BUILT is None:
        _BUILT = build_bass()
    nc = _BUILT

    bf = ml_dtypes.bfloat16
    q = np.asarray(q, np.float32)
    k = np.asarray(k, np.float32)
    v = np.asarray(v, np.float32)
    Wq = np.asarray(Wq, np.float32)
    Wk = np.asarray(Wk, np.float32)
    Wv = np.asarray(Wv, np.float32)
    Wo = np.asarray(Wo, np.float32)
    bo = np.asarray(bo, np.float32)
    mask = np.asarray(mask)

    cosT, sinT = _cos_sin_tables()
    ones64 = np.ones((1, 64), bf)
    ident = np.eye(128, dtype=ml_dtypes.float8_e5m2)
    perm = _rope_perm_cols()
    qTb = [_tile_xT(q[b].T.astype(bf)) for b in range(2)]
    kTb = [_tile_xT(k[b].T.astype(bf)) for b in range(2)]
    vTb = [_tile_vT(v[b].T.astype(bf)) for b in range(2)]
    mknegb = [_tile_mask(
        (MASK_NEG * (1.0 - mask[b, 0].T.astype(np.float32))).astype(
            ml_dtypes.float8_e5m2)) for b in range(2)]

    in_maps = []
    for c in range(N_CORES):
        b = c // 4
        head_base = (c % 4) * 4
        cols = slice(head_base * 64, head_base * 64 + 256)
        in_maps.append({
            "qT": qTb[b], "kT": kTb[b], "vT": vTb[b],
            "wq": _tile_w(Wq[:, cols][:, perm].astype(bf)),
            "wk": _tile_w(Wk[:, cols][:, perm].astype(bf)),
            "wv": _tile_w(Wv[:, cols].astype(bf)),
            "wo": np.ascontiguousarray(
                Wo[cols, :].reshape(4, 64, DIM).transpose(1, 0, 2).astype(bf)),
            "cosT": cosT, "sinT": sinT,
            "mkneg": mknegb[b], "ident": ident, "ones64": ones64,
        })

    kernel._last_in_maps = in_maps
    res = run_bass_kernel_spmd(nc, in_maps, core_ids=list(range(N_CORES)))
    global _last_res
    _last_res = res.results
    out = np.zeros((2, S, DIM), np.float32)
    for c in range(N_CORES):
        out[c // 4] += res.results[c]["out_part"].astype(np.float32)
    out += bo[None, None, :]
    return out


# revision 12
# speedup vs baseline: 1.7242x; 1.0633x over previous
"""Multi-head attention (RoPE + mask + softmax) Trainium2 Bass kernel, v3.

Sharding: 8 cores = 2 batches x 4 head-groups. Core c handles batch c//4,
local heads 4*(c%4) .. +4 (tensor-parallel on heads; Wq/Wk/Wv column-sharded,
Wo row-sharded; per-core partial outputs summed on host).

v3: all-fp16 datapath (bf16 lacked precision margin); scores row-tiled by
head pair (K=64 halves run concurrently via tile_position); mask folded into
PE as an identity-matmul accumulation of -16384*(1-mask) (fp8e5) for head
h2=0 (and both heads on q-block 0), multiplicative fp16 mask on DVE for
h2=1 otherwise; exp on ScalarE at N=1024 per (hp, m) with PSUM
double-buffering; attention m-loop software-pipelined (scores m+1 emitted
before attn@V m) so the ScalarE exp rate (~1.15us/iter) sets the pace;
v-proj upfront, out-proj chunks and normalization interleaved into PE/DVE
slack of the next q-block; RoPE pair-swap via DVE stream_shuffle (16-lane
sub-blocks) instead of SBUF-SBUF DMA.

Per-core layout: local head h = 2*hp + h2; qhT/khT [128, hp, s] with
h2=0 on partitions 0-63, h2=1 on 64-127. Within a head's 64 partitions:
[even dims 0-30 | odd 1-31 | even 32-62 | odd 33-63] (16 each) so the
RoPE partner swap is lane^16, expressible as a 32-lane stream_shuffle.
"""
import sys
sys.path.insert(0, '/opt/trn_rl_repo')
import math
import numpy as np
import ml_dtypes

import concourse.bass as bass
import concourse.mybir as mybir
import concourse.tile as tile
from concourse import bacc
from concourse.bass_utils import run_bass_kernel_spmd

F32 = mybir.dt.float32
F16 = mybir.dt.float16
F8E5 = mybir.dt.float8e5

S = 2048
DIM = 1024
HEAD_DIM = 64
N_CORES = 8
KC = DIM // 128          # 8 contraction chunks for projections
MT = S // 128            # 16 k-chunks in attention
QB = 512                 # q-block width
NQB = S // QB            # 4
ROPE_THETA = 10000.0
MASK_NEG = -16384.0      # exactly representable in fp8e5; (s-16384)/8 -> exp=0

SWAP16 = [i ^ 16 for i in range(32)]   # stream_shuffle mask: lane p <-> p^16

_BUILT = None
DEBUG_DUMPS = False


def build_bass():
    nc = bacc.Bacc("TRN2", target_bir_lowering=False, debug=False)

    qT = nc.dram_tensor("qT", [4, 128, KC, 512], F16, kind="ExternalInput").ap()
    kT = nc.dram_tensor("kT", [4, 128, KC, 512], F16, kind="ExternalInput").ap()
    vT = nc.dram_tensor("vT", [MT, 128, KC, 128], F16, kind="ExternalInput").ap()
    wq = nc.dram_tensor("wq", [128, KC, 256], F16, kind="ExternalInput").ap()
    wk = nc.dram_tensor("wk", [128, KC, 256], F16, kind="ExternalInput").ap()
    wv = nc.dram_tensor("wv", [128, KC, 256], F16, kind="ExternalInput").ap()
    wo = nc.dram_tensor("wo", [64, 4, DIM], F16, kind="ExternalInput").ap()
    cosT = nc.dram_tensor("cosT", [128, S], F16, kind="ExternalInput").ap()
    sinT = nc.dram_tensor("sinT", [128, S], F16, kind="ExternalInput").ap()
    mkneg = nc.dram_tensor("mkneg", [128, MT, S], F8E5, kind="ExternalInput").ap()
    ident = nc.dram_tensor("ident", [128, 128], F8E5, kind="ExternalInput").ap()
    ones64 = nc.dram_tensor("ones64", [1, 64], F16, kind="ExternalInput").ap()
    out_part = nc.dram_tensor("out_part", [S, DIM], F16, kind="ExternalOutput").ap()
    if DEBUG_DUMPS:
        dbg_qhT = nc.dram_tensor("dbg_qhT", [128, 2, S], F16,
                                 kind="ExternalOutput").ap()
        dbg_khT = nc.dram_tensor("dbg_khT", [128, 2, S], F16,
                                 kind="ExternalOutput").ap()
        dbg_vh = nc.dram_tensor("dbg_vh", [128, MT, 4 * 65], F16,
                                kind="ExternalOutput").ap()
        dbg_at = nc.dram_tensor("dbg_at", [8, 128, 1024], F16,
                                kind="ExternalOutput").ap()
        dbg_outT2 = nc.dram_tensor("dbg_outT2", [64, 4, S], F16,
                                   kind="ExternalOutput").ap()
        dbg_outT2n = nc.dram_tensor("dbg_outT2n", [64, 4, S], F16,
                                    kind="ExternalOutput").ap()

    with tile.TileContext(nc) as tc:
        with tc.tile_pool(name="persist", bufs=1) as persist, \
             tc.tile_pool(name="dram", bufs=1, space="DRAM") as dram, \
             tc.tile_pool(name="ps_sc", bufs=2, space="PSUM") as ps_sc, \
             tc.tile_pool(name="ps_avp", bufs=1, space="PSUM") as ps_avp, \
             tc.tile_pool(name="ps_misc", bufs=2, space="PSUM") as ps_misc:

            qhT = persist.tile([128, 2, S], F16)     # [dim-part, hp, s]
            khT = persist.tile([128, 2, S], F16)
            vh = persist.tile([128, MT, 4 * 65], F16)
            outT2 = persist.tile([64, 4, S], F16)
            wo_sb = persist.tile([64, 4, DIM], F16)
            ones_sb = persist.tile([1, 64], F16)
            id_sb = persist.tile([128, 128], F8E5)
            cos_sb = persist.tile([128, S], F16)
            sin_sb = persist.tile([128, S], F16)
            mkneg_sb = persist.tile([128, MT, S], F8E5)
            mkmult_sb = persist.tile([128, MT, S], F16)
            dscr = dram.tile([2 * NQB, 2 * QB], F32)
            dscr2 = dram.tile([2 * NQB, 2 * QB], F16)

            nc.sync.dma_start(out=wo_sb, in_=wo)
            nc.sync.dma_start(out=ones_sb, in_=ones64)
            nc.sync.dma_start(out=id_sb, in_=ident)
            nc.sync.dma_start(out=cos_sb, in_=cosT)
            nc.sync.dma_start(out=sin_sb, in_=sinT)
            # ones column for the denominator rows of vh
            nc.vector.memset(
                vh.rearrange("p m (h x) -> p m h x", x=65)[:, :, :, 64:65], 1.0)

            # ---------------- Phase 1: q/k projections + RoPE ----------------
            with tc.tile_pool(name="proj", bufs=1) as projp, \
                 tc.tile_pool(name="xts", bufs=3) as xts, \
                 tc.tile_pool(name="rope", bufs=3) as rope:
                wq_sb = projp.tile([128, KC, 256], F16)
                wk_sb = projp.tile([128, KC, 256], F16)
                wv_sb = projp.tile([128, KC, 256], F16)
                nc.sync.dma_start(out=wq_sb, in_=wq)
                nc.sync.dma_start(out=wk_sb, in_=wk)
                nc.sync.dma_start(out=wv_sb, in_=wv)

                for xdram, w_sb, dstT in ((qT, wq_sb, qhT), (kT, wk_sb, khT)):
                    for sblk in range(4):
                        x_sb = xts.tile([128, KC, 512], F16, tag="xts")
                        nc.sync.dma_start(out=x_sb, in_=xdram[sblk])
                        ss = slice(sblk * 512, (sblk + 1) * 512)
                        for hp in range(2):
                            psum = ps_sc.tile([128, 1024], F32, tag="sc")
                            for kc in range(KC):
                                nc.tensor.matmul(
                                    psum[:, 0:512],
                                    lhsT=w_sb[:, kc, hp * 128:(hp + 1) * 128],
                                    rhs=x_sb[:, kc, :],
                                    start=(kc == 0), stop=(kc == KC - 1))
                            qraw = rope.tile([128, 512], F16, tag="qraw")
                            nc.vector.tensor_copy(qraw, psum[:, 0:512])
                            t = rope.tile([128, 512], F16, tag="t")
                            u = rope.tile([128, 512], F16, tag="u")
                            nc.vector.tensor_mul(t, qraw, cos_sb[:, ss])
                            nc.vector.tensor_mul(u, qraw, sin_sb[:, ss])
                            us = rope.tile([128, 512], F16, tag="us")
                            nc.vector.stream_shuffle(us, u, SWAP16)
                            nc.vector.tensor_add(dstT[:, hp, ss], t, us)

                # ---------------- Phase 2: v projection ----------------
                # (mkneg chunks interleaved with v tiles so the mask stream
                # trails v in the DMA queue but stays ahead of attention use)
                for sc in range(4):
                    nc.sync.dma_start(out=mkneg_sb[:, sc * 2:sc * 2 + 2, :],
                                      in_=mkneg[:, sc * 2:sc * 2 + 2, :])
                for sc in range(MT):
                    v_sb = xts.tile([128, KC, 128], F16, tag="xts")
                    nc.sync.dma_start(out=v_sb, in_=vT[sc])
                    if sc * 2 + 8 + 1 < MT:
                        m0 = 8 + sc * 2
                        nc.sync.dma_start(out=mkneg_sb[:, m0:m0 + 2, :],
                                          in_=mkneg[:, m0:m0 + 2, :])
                    psum = ps_misc.tile([128, 512], F32, tag="misc")
                    for kc in range(KC):
                        nc.tensor.matmul(
                            psum[:, 0:256], lhsT=v_sb[:, kc, :], rhs=wv_sb[:, kc, :],
                            start=(kc == 0), stop=(kc == KC - 1))
                    nc.scalar.copy(
                        vh[:, sc, :].rearrange("p (h x) -> p h x", x=65)[:, :, 0:64],
                        psum[:, 0:256].rearrange("p (h x) -> p h x", x=64))

            # ---------------- Phase 3: attention ----------------
            with tc.tile_pool(name="at", bufs=3) as atp, \
                 tc.tile_pool(name="atm", bufs=3) as atmp, \
                 tc.tile_pool(name="stg", bufs=2) as stgp, \
                 tc.tile_pool(name="dn", bufs=2) as dnp, \
                 tc.tile_pool(name="co", bufs=3) as cop:

                def outproj_chunk(qbo, j):
                    # out-projection chunk j (of 8) for q-block qbo
                    sci, nb = qbo * 4 + j // 2, j % 2
                    n5 = slice(nb * 512, (nb + 1) * 512)
                    wps = ps_misc.tile([128, 512], F32, tag="misc")
                    for h in range(4):
                        nc.tensor.matmul(
                            wps,
                            lhsT=outT2[0:64, h, sci * 128:(sci + 1) * 128],
                            rhs=wo_sb[0:64, h, n5],
                            start=(h == 0), stop=(h == 3))
                    co = cop.tile([128, 512], F16, tag="co")
                    nc.vector.tensor_copy(co, wps)
                    nc.sync.dma_start(
                        out=out_part[sci * 128:(sci + 1) * 128, n5], in_=co)

                for qb in range(NQB):
                    qs = slice(qb * QB, (qb + 1) * QB)
                    for hp in range(2):
                        avp = ps_avp.tile([128, 1024], F32, tag="avp")

                        def emit_scores(m):
                            # scores for both heads of the pair + PE mask
                            sct = ps_sc.tile([128, 1024], F32, tag="sc")
                            nc.tensor.matmul(
                                sct[:, 0:512],
                                lhsT=khT[0:64, hp, m * 128:(m + 1) * 128],
                                rhs=qhT[0:64, hp, qs],
                                start=True, stop=False, tile_position=(0, 0))
                            nc.tensor.matmul(
                                sct[:, 512:1024],
                                lhsT=khT[64:128, hp, m * 128:(m + 1) * 128],
                                rhs=qhT[64:128, hp, qs],
                                start=True, stop=(qb > 0), tile_position=(64, 0))
                            nc.tensor.matmul(
                                sct[:, 0:512], lhsT=id_sb,
                                rhs=mkneg_sb[:, m, qs],
                                start=False, stop=True)
                            if qb == 0:
                                nc.tensor.matmul(
                                    sct[:, 512:1024], lhsT=id_sb,
                                    rhs=mkneg_sb[:, m, qs],
                                    start=False, stop=True)
                            return sct

                        sct = emit_scores(0)
                        for m in range(MT):
                            it = hp * MT + m
                            at = atp.tile([128, 1024], F16, tag="at")
                            nc.scalar.activation(
                                at, sct, mybir.ActivationFunctionType.Exp,
                                scale=1.0 / math.sqrt(HEAD_DIM))
                            if DEBUG_DUMPS and qb == 0 and hp == 0 and m < 8:
                                nc.sync.dma_start(out=dbg_at[m], in_=at)
                            # software-pipeline: next scores before attn@V(m)
                            if m + 1 < MT:
                                sct = emit_scores(m + 1)
                            if qb == 0 and hp == 0:
                                # decode multiplicative mask: 1 + mkneg/16384
                                # (mkneg is -16384 at masked positions, 0 else)
                                nc.vector.tensor_scalar(
                                    mkmult_sb[:, m, :], mkneg_sb[:, m, :],
                                    1.0 / 16384.0, 1.0,
                                    op0=mybir.AluOpType.mult,
                                    op1=mybir.AluOpType.add)
                            if qb > 0:
                                atm1 = atmp.tile([128, 512], F16, tag="atm")
                                nc.vector.tensor_mul(
                                    atm1, at[:, 512:1024], mkmult_sb[:, m, qs])
                                rhs1 = atm1
                            else:
                                rhs1 = at[:, 512:1024]
                            # attn @ V (K=128, M=65 with denominator column)
                            h0, h1 = 2 * hp, 2 * hp + 1
                            nc.tensor.matmul(
                                avp[0:65, 0:512],
                                lhsT=vh[:, m, h0 * 65:(h0 + 1) * 65],
                                rhs=at[:, 0:512],
                                start=(m == 0), stop=(m == MT - 1))
                            nc.tensor.matmul(
                                avp[0:65, 512:1024],
                                lhsT=vh[:, m, h1 * 65:(h1 + 1) * 65],
                                rhs=rhs1,
                                start=(m == 0), stop=(m == MT - 1))
                            # interleave prev q-block's out-projection chunks
                            if qb > 0 and it >= 8 and (it - 8) % 3 == 0 \
                                    and (it - 8) // 3 < 8:
                                outproj_chunk(qb - 1, (it - 8) // 3)
                        # evict this (qb, hp): data rows + denominator row
                        for h2 in range(2):
                            nc.vector.tensor_copy(
                                outT2[0:64, 2 * hp + h2, qs],
                                avp[0:64, h2 * 512:(h2 + 1) * 512])
                        if DEBUG_DUMPS:
                            for h2 in range(2):
                                nc.sync.dma_start(
                                    out=dbg_outT2[:, 2 * hp + h2, qs],
                                    in_=outT2[0:64, 2 * hp + h2, qs])
                        stg = stgp.tile([128, 1024], F32, tag="stg")
                        nc.vector.tensor_copy(stg[64:65, :], avp[64:65, :])
                        nc.sync.dma_start(out=dscr[qb * 2 + hp, :],
                                          in_=stg[64:65, :])
                    # denominators for this q-block: bounce, reciprocal, norm
                    rin = dnp.tile([128, 2, 8], F32, tag="rin")
                    nc.sync.dma_start(
                        out=rin,
                        in_=dscr[qb * 2:qb * 2 + 2].rearrange(
                            "u (p f) -> p u f", p=128))
                    r32 = dnp.tile([128, 2, 8], F32, tag="r32")
                    scr = dnp.tile([128, 2, 8], F32, tag="scr")
                    nc.vector.reciprocal_approx_accurate(r32, rin, scr)
                    rr = dnp.tile([128, 2, 8], F16, tag="rr")
                    nc.vector.tensor_copy(rr, r32)
                    nc.sync.dma_start(
                        out=dscr2[qb * 2:qb * 2 + 2].rearrange(
                            "u (p f) -> p u f", p=128),
                        in_=rr)
                    for hp in range(2):
                        for h2 in range(2):
                            h = 2 * hp + h2
                            rdn = dnp.tile([1, 512], F16, tag="rdn")
                            nc.sync.dma_start(
                                out=rdn,
                                in_=dscr2[qb * 2 + hp:qb * 2 + hp + 1,
                                          h2 * 512:(h2 + 1) * 512])
                            pbc = ps_misc.tile([128, 512], F32, tag="misc")
                            nc.tensor.matmul(pbc[0:64, :], lhsT=ones_sb,
                                             rhs=rdn, start=True, stop=True)
                            pbcb = dnp.tile([64, 512], F16, tag="pbcb")
                            nc.vector.tensor_copy(pbcb, pbc[0:64, :])
                            nc.vector.tensor_mul(outT2[0:64, h, qs],
                                                 outT2[0:64, h, qs], pbcb)
                # tail: out-projection for the last q-block
                for j in range(8):
                    outproj_chunk(NQB - 1, j)
                if DEBUG_DUMPS:
                    nc.sync.dma_start(out=dbg_qhT, in_=qhT)
                    nc.sync.dma_start(out=dbg_khT, in_=khT)
                    nc.sync.dma_start(out=dbg_vh, in_=vh)
                    nc.sync.dma_start(out=dbg_outT2n, in_=outT2)

    nc.compile()
    return nc


def _rope_perm_cols():
    """Column permutation of the 256-wide W slice for one core's 4 heads.

    Chunk hp (0,1) holds local heads 2hp, 2hp+1 as partition halves.
    Within a head's 64 rows: [even 0-30 | odd 1-31 | even 32-62 | odd 33-63]
    (16 each) so the RoPE partner is at lane^16.
    """
    cols = []
    for c in range(2):
        for j2 in range(2):          # which head within the chunk
            head = 2 * c + j2
            for half in range(2):    # dims 0-31, dims 32-63
                for par in range(2):     # even (x1), odd (x2)
                    for i in range(16):
                        cols.append(head * 64 + half * 32 + 2 * i + par)
    return np.array(cols)


def _cos_sin_tables():
    inv_freq = 1.0 / (ROPE_THETA ** (np.arange(0, HEAD_DIM, 2, dtype=np.float64)
                                     / HEAD_DIM))          # [32]
    ang = np.arange(S, dtype=np.float64)[None, :] * inv_freq[:, None]  # [32, S]
    cos32 = np.cos(ang)
    sin32 = np.sin(ang)
    # row blocks of 16 per 64-row head: [c0:16, c0:16, c16:32, c16:32]
    cos64 = np.concatenate([cos32[0:16], cos32[0:16],
                            cos32[16:32], cos32[16:32]], axis=0)
    sin64 = np.concatenate([sin32[0:16], -sin32[0:16],
                            sin32[16:32], -sin32[16:32]], axis=0)
    cosT = np.tile(cos64, (2, 1)).astype(np.float16)        # [128, S]
    sinT = np.tile(sin64, (2, 1)).astype(np.float16)
    return np.ascontiguousarray(cosT), np.ascontiguousarray(sinT)


def _tile_xT(xT):
    # [1024, 2048] -> [4 sblk, 128 part, 8 kc, 512]
    return np.ascontiguousarray(
        xT.reshape(KC, 128, 4, 512).transpose(2, 1, 0, 3))


def _tile_vT(vT):
    # [1024, 2048] -> [16 sc, 128 part, 8 kc, 128]
    return np.ascontiguousarray(
        vT.reshape(KC, 128, MT, 128).transpose(2, 1, 0, 3))


def _tile_w(w):
    # [1024, 256] -> [128, 8, 256]
    return np.ascontiguousarray(w.reshape(KC, 128, 256).transpose(1, 0, 2))


def _tile_mask(maskT):
    # [2048 k, 2048 q] -> [128, 16 m, 2048]
    return np.ascontiguousarray(
        maskT.reshape(MT, 128, S).transpose(1, 0, 2))


def kernel(q, k, v, mask, Wq, Wk, Wv, Wo, bo):
    global _BUILT
    if _BUILT is None:
        _BUILT = build_bass()
    nc = _BUILT

    f16 = np.float16
    q = np.asarray(q, np.float32)
    k = np.asarray(k, np.float32)
    v = np.asarray(v, np.float32)
    Wq = np.asarray(Wq, np.float32)
    Wk = np.asarray(Wk, np.float32)
    Wv = np.asarray(Wv, np.float32)
    Wo = np.asarray(Wo, np.float32)
    bo = np.asarray(bo, np.float32)
    mask = np.asarray(mask)

    cosT, sinT = _cos_sin_tables()
    ones64 = np.ones((1, 64), f16)
    ident = np.eye(128, dtype=ml_dtypes.float8_e5m2)
    perm = _rope_perm_cols()
    qTb = [_tile_xT(q[b].T.astype(f16)) for b in range(2)]
    kTb = [_tile_xT(k[b].T.astype(f16)) for b in range(2)]
    vTb = [_tile_vT(v[b].T.astype(f16)) for b in range(2)]
    mknegb = [_tile_mask(
        (MASK_NEG * (1.0 - mask[b, 0].T.astype(np.float32))).astype(
            ml_dtypes.float8_e5m2)) for b in range(2)]

    in_maps = []
    for c in range(N_CORES):
        b = c // 4
        head_base = (c % 4) * 4
        cols = slice(head_base * 64, head_base * 64 + 256)
        in_maps.append({
            "qT": qTb[b], "kT": kTb[b], "vT": vTb[b],
            "wq": _tile_w(Wq[:, cols][:, perm].astype(f16)),
            "wk": _tile_w(Wk[:, cols][:, perm].astype(f16)),
            "wv": _tile_w(Wv[:, cols].astype(f16)),
            "wo": np.ascontiguousarray(
                Wo[cols, :].reshape(4, 64, DIM).transpose(1, 0, 2).astype(f16)),
            "cosT": cosT, "sinT": sinT,
            "mkneg": mknegb[b], "ident": ident, "ones64": ones64,
        })

    kernel._last_in_maps = in_maps
    res = run_bass_kernel_spmd(nc, in_maps, core_ids=list(range(N_CORES)))
    global _last_res
    _last_res = res.results
    out = np.zeros((2, S, DIM), np.float32)
    for c in range(N_CORES):
        out[c // 4] += res.results[c]["out_part"].astype(np.float32)
    out += bo[None, None, :]
    return out


# revision 14
# speedup vs baseline: 1.8568x; 1.0769x over previous
"""Multi-head attention (RoPE + mask + softmax) Trainium2 Bass kernel, v4.

Sharding: 8 cores = 2 batches x 4 head-groups. Core c handles batch c//4,
local heads 4*(c%4) .. +4 (tensor-parallel on heads; Wq/Wk/Wv column-sharded,
Wo row-sharded; per-core partial outputs summed on host).

v4: all-fp16 datapath. Scores row-tiled by head pair (K=64 halves run
concurrently). Masking is multiplicative on DVE for both heads (mask decoded
from fp8e5 -16384*(1-mask) to fp16 {0,1} on the otherwise-idle ScalarE
during phase 1). exp on ScalarE at N=1024 per (hp, m) with PSUM
double-buffering; attention m-loop software-pipelined one iteration ahead,
including across (qb, hp) block boundaries so ScalarE never gaps.
Denominator reciprocal (DRAM bounce) and normalization are emitted into
fixed slots of the NEXT q-block's m-loop so no engine stream blocks on the
bounce latency; out-projection uses K=128 head-pair matmuls (heads stacked
on 128 partitions; the upper half is lane-shifted via a SBUF->SBUF gpsimd
DMA after normalization).
"""
import sys
sys.path.insert(0, '/opt/trn_rl_repo')
import math
import numpy as np
import ml_dtypes

import concourse.bass as bass
import concourse.mybir as mybir
import concourse.tile as tile
from concourse import bacc
from concourse.bass_utils import run_bass_kernel_spmd

F32 = mybir.dt.float32
F16 = mybir.dt.float16
F8E5 = mybir.dt.float8e5

S = 2048
DIM = 1024
HEAD_DIM = 64
N_CORES = 8
KC = DIM // 128          # 8 contraction chunks for projections
MT = S // 128            # 16 k-chunks in attention
QB = 512                 # q-block width
NQB = S // QB            # 4
ROPE_THETA = 10000.0
MASK_NEG = -16384.0      # exactly representable in fp8e5

SWAP16 = [i ^ 16 for i in range(32)]   # stream_shuffle mask: lane p <-> p^16

_BUILT = None
DEBUG_DUMPS = False


def build_bass():
    nc = bacc.Bacc("TRN2", target_bir_lowering=False, debug=False)

    qT = nc.dram_tensor("qT", [4, 128, KC, 512], F16, kind="ExternalInput").ap()
    kT = nc.dram_tensor("kT", [4, 128, KC, 512], F16, kind="ExternalInput").ap()
    vT = nc.dram_tensor("vT", [MT, 128, KC, 128], F16, kind="ExternalInput").ap()
    wq = nc.dram_tensor("wq", [128, KC, 256], F16, kind="ExternalInput").ap()
    wk = nc.dram_tensor("wk", [128, KC, 256], F16, kind="ExternalInput").ap()
    wv = nc.dram_tensor("wv", [128, KC, 256], F16, kind="ExternalInput").ap()
    wo2 = nc.dram_tensor("wo2", [128, 2, DIM], F16, kind="ExternalInput").ap()
    cosT = nc.dram_tensor("cosT", [128, S], F16, kind="ExternalInput").ap()
    sinT = nc.dram_tensor("sinT", [128, S], F16, kind="ExternalInput").ap()
    mkneg = nc.dram_tensor("mkneg", [128, MT, S], F8E5, kind="ExternalInput").ap()
    ones64 = nc.dram_tensor("ones64", [1, 64], F16, kind="ExternalInput").ap()
    out_part = nc.dram_tensor("out_part", [S, DIM], F16, kind="ExternalOutput").ap()
    if DEBUG_DUMPS:
        dbg_qhT = nc.dram_tensor("dbg_qhT", [128, 2, S], F16,
                                 kind="ExternalOutput").ap()
        dbg_khT = nc.dram_tensor("dbg_khT", [128, 2, S], F16,
                                 kind="ExternalOutput").ap()
        dbg_vh = nc.dram_tensor("dbg_vh", [128, MT, 4 * 65], F16,
                                kind="ExternalOutput").ap()
        dbg_at = nc.dram_tensor("dbg_at", [8, 128, 1024], F16,
                                kind="ExternalOutput").ap()
        dbg_outT2 = nc.dram_tensor("dbg_outT2", [128, 2, S], F16,
                                   kind="ExternalOutput").ap()

    with tile.TileContext(nc) as tc:
        with tc.tile_pool(name="persist", bufs=1) as persist, \
             tc.tile_pool(name="dram", bufs=1, space="DRAM") as dram, \
             tc.tile_pool(name="ps_sc", bufs=2, space="PSUM") as ps_sc, \
             tc.tile_pool(name="ps_avp", bufs=1, space="PSUM") as ps_avp, \
             tc.tile_pool(name="ps_misc", bufs=2, space="PSUM") as ps_misc:

            qhT = persist.tile([128, 2, S], F16)     # [dim-part, hp, s]
            khT = persist.tile([128, 2, S], F16)
            vh = persist.tile([128, MT, 4 * 65], F16)
            # heads stacked for K=128 out-proj: partitions 0-63 head 2hp,
            # 64-127 head 2hp+1; slot dim = hp
            outT2 = persist.tile([128, 2, S], F16)
            wo_sb = persist.tile([128, 2, DIM], F16)
            ones_sb = persist.tile([1, 64], F16)
            cos_sb = persist.tile([128, S], F16)
            sin_sb = persist.tile([128, S], F16)
            mkneg_sb = persist.tile([128, MT, S], F8E5)
            mkmult_sb = persist.tile([128, MT, S], F16)
            dscr = dram.tile([2 * NQB, 2 * QB], F32)
            dscr2 = dram.tile([2 * NQB, 2 * QB], F16)

            nc.vector.memset(
                vh.rearrange("p m (h x) -> p m h x", x=65)[:, :, :, 64:65], 1.0)

            # ---------------- Phase 1: q/k projections + RoPE ----------------
            with tc.tile_pool(name="proj", bufs=1) as projp, \
                 tc.tile_pool(name="xts", bufs=3) as xts, \
                 tc.tile_pool(name="rope", bufs=3) as rope:
                wq_sb = projp.tile([128, KC, 256], F16)
                wk_sb = projp.tile([128, KC, 256], F16)
                wv_sb = projp.tile([128, KC, 256], F16)
                nc.sync.dma_start(out=wq_sb, in_=wq)
                nc.sync.dma_start(out=wk_sb, in_=wk)

                first = True
                for xdram, w_sb, dstT in ((qT, wq_sb, qhT), (kT, wk_sb, khT)):
                    for sblk in range(4):
                        x_sb = xts.tile([128, KC, 512], F16, tag="xts")
                        nc.sync.dma_start(out=x_sb, in_=xdram[sblk])
                        if first:
                            nc.sync.dma_start(out=cos_sb, in_=cosT)
                            nc.sync.dma_start(out=sin_sb, in_=sinT)
                            nc.sync.dma_start(out=wv_sb, in_=wv)
                            first = False
                        ss = slice(sblk * 512, (sblk + 1) * 512)
                        for hp in range(2):
                            psum = ps_sc.tile([128, 1024], F32, tag="sc")
                            for kc in range(KC):
                                nc.tensor.matmul(
                                    psum[:, 0:512],
                                    lhsT=w_sb[:, kc, hp * 128:(hp + 1) * 128],
                                    rhs=x_sb[:, kc, :],
                                    start=(kc == 0), stop=(kc == KC - 1))
                            qraw = rope.tile([128, 512], F16, tag="qraw")
                            nc.scalar.copy(qraw, psum[:, 0:512])
                            t = rope.tile([128, 512], F16, tag="t")
                            u = rope.tile([128, 512], F16, tag="u")
                            nc.vector.tensor_mul(t, qraw, cos_sb[:, ss])
                            nc.vector.tensor_mul(u, qraw, sin_sb[:, ss])
                            us = rope.tile([128, 512], F16, tag="us")
                            nc.vector.stream_shuffle(us, u, SWAP16)
                            nc.vector.tensor_add(dstT[:, hp, ss], t, us)

                # ---------------- Phase 2: v projection + mask decode -------
                for g in range(8):
                    nc.sync.dma_start(out=mkneg_sb[:, g * 2:g * 2 + 2, :],
                                      in_=mkneg[:, g * 2:g * 2 + 2, :])
                for sc in range(MT):
                    v_sb = xts.tile([128, KC, 128], F16, tag="xts")
                    nc.sync.dma_start(out=v_sb, in_=vT[sc])
                    psum = ps_misc.tile([128, 512], F32, tag="misc")
                    for kc in range(KC):
                        nc.tensor.matmul(
                            psum[:, 0:256], lhsT=v_sb[:, kc, :], rhs=wv_sb[:, kc, :],
                            start=(kc == 0), stop=(kc == KC - 1))
                    nc.scalar.copy(
                        vh[:, sc, :].rearrange("p (h x) -> p h x", x=65)[:, :, 0:64],
                        psum[:, 0:256].rearrange("p (h x) -> p h x", x=64))
                    if sc < 8:
                        # decode multiplicative mask on idle ScalarE:
                        # 1 + mkneg/16384 -> {0, 1} fp16
                        nc.scalar.activation(
                            mkmult_sb[:, sc * 2:sc * 2 + 2, :],
                            mkneg_sb[:, sc * 2:sc * 2 + 2, :],
                            mybir.ActivationFunctionType.Identity,
                            scale=1.0 / 16384.0, bias=1.0)
                nc.sync.dma_start(out=wo_sb, in_=wo2)
                nc.sync.dma_start(out=ones_sb, in_=ones64)

            # ---------------- Phase 3: attention ----------------
            with tc.tile_pool(name="at", bufs=3) as atp, \
                 tc.tile_pool(name="atm", bufs=3) as atmp, \
                 tc.tile_pool(name="stg", bufs=2) as stgp, \
                 tc.tile_pool(name="dn", bufs=4) as dnp, \
                 tc.tile_pool(name="tmp", bufs=4) as tmpp, \
                 tc.tile_pool(name="co", bufs=3) as cop:

                tmps = {}

                def emit_scores(qb, hp, m):
                    qs = slice(qb * QB, (qb + 1) * QB)
                    sct = ps_sc.tile([128, 1024], F32, tag="sc")
                    nc.tensor.matmul(
                        sct[:, 0:512],
                        lhsT=khT[0:64, hp, m * 128:(m + 1) * 128],
                        rhs=qhT[0:64, hp, qs],
                        start=True, stop=True, tile_position=(0, 0))
                    nc.tensor.matmul(
                        sct[:, 512:1024],
                        lhsT=khT[64:128, hp, m * 128:(m + 1) * 128],
                        rhs=qhT[64:128, hp, qs],
                        start=True, stop=True, tile_position=(64, 0))
                    return sct

                def outproj_chunk(qbo, j):
                    # out-projection chunk j (of 8) for q-block qbo; K=128
                    sci, nb = qbo * 4 + j // 2, j % 2
                    n5 = slice(nb * 512, (nb + 1) * 512)
                    wps = ps_misc.tile([128, 512], F32, tag="misc")
                    for sl in range(2):
                        nc.tensor.matmul(
                            wps,
                            lhsT=outT2[:, sl, sci * 128:(sci + 1) * 128],
                            rhs=wo_sb[:, sl, n5],
                            start=(sl == 0), stop=(sl == 1))
                    co = cop.tile([128, 512], F16, tag="co")
                    nc.vector.tensor_copy(co, wps)
                    nc.sync.dma_start(
                        out=out_part[sci * 128:(sci + 1) * 128, n5], in_=co)

                def finish_qb(qbp, it):
                    """Deferred denominator/normalize/out-proj for q-block qbp,
                    scheduled at slot `it` (0..19) of the following q-block."""
                    qsp = slice(qbp * QB, (qbp + 1) * QB)
                    if it == 2:
                        rin = dnp.tile([128, 2, 8], F32, tag="rin")
                        nc.sync.dma_start(
                            out=rin,
                            in_=dscr[qbp * 2:qbp * 2 + 2].rearrange(
                                "u (p f) -> p u f", p=128))
                        finish_qb.rin = rin
                    elif it == 4:
                        r32 = dnp.tile([128, 2, 8], F32, tag="r32")
                        scr = dnp.tile([128, 2, 8], F32, tag="scr")
                        nc.vector.reciprocal_approx_accurate(
                            r32, finish_qb.rin, scr)
                        rr = dnp.tile([128, 2, 8], F16, tag="rr")
                        nc.vector.tensor_copy(rr, r32)
                        nc.sync.dma_start(
                            out=dscr2[qbp * 2:qbp * 2 + 2].rearrange(
                                "u (p f) -> p u f", p=128),
                            in_=rr)
                    elif it == 6:
                        finish_qb.rdn = {}
                        for hpp in range(2):
                            for h2 in range(2):
                                rdn = dnp.tile([1, 512], F16, tag="rdn")
                                nc.sync.dma_start(
                                    out=rdn,
                                    in_=dscr2[qbp * 2 + hpp:qbp * 2 + hpp + 1,
                                              h2 * 512:(h2 + 1) * 512])
                                finish_qb.rdn[(hpp, h2)] = rdn
                    elif 8 <= it < 12:
                        u = it - 8
                        hpp, h2 = u // 2, u % 2
                        pbc = ps_misc.tile([128, 512], F32, tag="misc")
                        nc.tensor.matmul(pbc[0:64, :], lhsT=ones_sb,
                                         rhs=finish_qb.rdn[(hpp, h2)],
                                         start=True, stop=True)
                        pbcb = dnp.tile([64, 512], F16, tag="pbcb")
                        nc.vector.tensor_copy(pbcb, pbc[0:64, :])
                        if h2 == 0:
                            nc.vector.tensor_mul(outT2[0:64, hpp, qsp],
                                                 outT2[0:64, hpp, qsp], pbcb)
                        else:
                            tmpt = tmps.pop((qbp, hpp))
                            nc.vector.tensor_mul(tmpt, tmpt, pbcb)
                            # lane-shift to partitions 64-127 for K=128 outproj
                            nc.gpsimd.dma_start(
                                out=outT2[64:128, hpp, qsp], in_=tmpt)
                    elif 12 <= it < 20:
                        outproj_chunk(qbp, it - 12)

                blocks = [(qb, hp) for qb in range(NQB) for hp in range(2)]
                sct = emit_scores(0, 0, 0)
                for bi, (qb, hp) in enumerate(blocks):
                    qs = slice(qb * QB, (qb + 1) * QB)
                    avp = ps_avp.tile([128, 1024], F32, tag="avp")
                    for m in range(MT):
                        at = atp.tile([128, 1024], F16, tag="at")
                        nc.scalar.activation(
                            at, sct, mybir.ActivationFunctionType.Exp,
                            scale=1.0 / math.sqrt(HEAD_DIM))
                        if DEBUG_DUMPS and bi == 0 and m < 8:
                            nc.sync.dma_start(out=dbg_at[m], in_=at)
                        # software-pipeline: next scores (possibly next block's)
                        if m + 1 < MT:
                            sct = emit_scores(qb, hp, m + 1)
                        elif bi + 1 < len(blocks):
                            nqb, nhp = blocks[bi + 1]
                            sct = emit_scores(nqb, nhp, 0)
                        # multiplicative mask, both heads, on DVE
                        atm = atmp.tile([128, 1024], F16, tag="atm")
                        nc.vector.tensor_mul(
                            atm[:, 0:512], at[:, 0:512], mkmult_sb[:, m, qs])
                        nc.vector.tensor_mul(
                            atm[:, 512:1024], at[:, 512:1024],
                            mkmult_sb[:, m, qs])
                        # attn @ V (K=128, M=65 with denominator column)
                        h0, h1 = 2 * hp, 2 * hp + 1
                        nc.tensor.matmul(
                            avp[0:65, 0:512],
                            lhsT=vh[:, m, h0 * 65:(h0 + 1) * 65],
                            rhs=atm[:, 0:512],
                            start=(m == 0), stop=(m == MT - 1))
                        nc.tensor.matmul(
                            avp[0:65, 512:1024],
                            lhsT=vh[:, m, h1 * 65:(h1 + 1) * 65],
                            rhs=atm[:, 512:1024],
                            start=(m == 0), stop=(m == MT - 1))
                        # deferred work for the previous q-block
                        if qb > 0:
                            finish_qb(qb - 1, hp * MT + m)
                    # evict this (qb, hp): head h2=0 rows to outT2 lower half,
                    # h2=1 to a staging tile (normalized there, then DMA'd to
                    # the upper half); denominator row to DRAM via stg
                    nc.vector.tensor_copy(outT2[0:64, hp, qs],
                                          avp[0:64, 0:512])
                    tmpt = tmpp.tile([64, 512], F16, tag="tmp")
                    nc.vector.tensor_copy(tmpt, avp[0:64, 512:1024])
                    tmps[(qb, hp)] = tmpt
                    stg = stgp.tile([128, 1024], F32, tag="stg")
                    nc.vector.tensor_copy(stg[64:65, :], avp[64:65, :])
                    nc.sync.dma_start(out=dscr[qb * 2 + hp, :],
                                      in_=stg[64:65, :])
                # tail: finish the last q-block
                for it in (2, 4, 6, 8, 9, 10, 11, 12, 13, 14, 15, 16, 17, 18, 19):
                    finish_qb(NQB - 1, it)
                if DEBUG_DUMPS:
                    nc.sync.dma_start(out=dbg_qhT, in_=qhT)
                    nc.sync.dma_start(out=dbg_khT, in_=khT)
                    nc.sync.dma_start(out=dbg_vh, in_=vh)
                    nc.sync.dma_start(out=dbg_outT2, in_=outT2)

    nc.compile()
    return nc


def _rope_perm_cols():
    """Column permutation of the 256-wide W slice for one core's 4 heads.

    Chunk hp (0,1) holds local heads 2hp, 2hp+1 as partition halves.
    Within a head's 64 rows: [even 0-30 | odd 1-31 | even 32-62 | odd 33-63]
    (16 each) so the RoPE partner is at lane^16.
    """
    cols = []
    for c in range(2):
        for j2 in range(2):          # which head within the chunk
            head = 2 * c + j2
            for half in range(2):    # dims 0-31, dims 32-63
                for par in range(2):     # even (x1), odd (x2)
                    for i in range(16):
                        cols.append(head * 64 + half * 32 + 2 * i + par)
    return np.array(cols)


def _cos_sin_tables():
    inv_freq = 1.0 / (ROPE_THETA ** (np.arange(0, HEAD_DIM, 2, dtype=np.float64)
                                     / HEAD_DIM))          # [32]
    ang = np.arange(S, dtype=np.float64)[None, :] * inv_freq[:, None]  # [32, S]
    cos32 = np.cos(ang)
    sin32 = np.sin(ang)
    # row blocks of 16 per 64-row head: [c0:16, c0:16, c16:32, c16:32]
    cos64 = np.concatenate([cos32[0:16], cos32[0:16],
                            cos32[16:32], cos32[16:32]], axis=0)
    sin64 = np.concatenate([sin32[0:16], -sin32[0:16],
                            sin32[16:32], -sin32[16:32]], axis=0)
    cosT = np.tile(cos64, (2, 1)).astype(np.float16)        # [128, S]
    sinT = np.tile(sin64, (2, 1)).astype(np.float16)
    return np.ascontiguousarray(cosT), np.ascontiguousarray(sinT)


def _tile_xT(xT):
    # [1024, 2048] -> [4 sblk, 128 part, 8 kc, 512]
    return np.ascontiguousarray(
        xT.reshape(KC, 128, 4, 512).transpose(2, 1, 0, 3))


def _tile_vT(vT):
    # [1024, 2048] -> [16 sc, 128 part, 8 kc, 128]
    return np.ascontiguousarray(
        vT.reshape(KC, 128, MT, 128).transpose(2, 1, 0, 3))


def _tile_w(w):
    # [1024, 256] -> [128, 8, 256]
    return np.ascontiguousarray(w.reshape(KC, 128, 256).transpose(1, 0, 2))


def _tile_mask(maskT):
    # [2048 k, 2048 q] -> [128, 16 m, 2048]
    return np.ascontiguousarray(
        maskT.reshape(MT, 128, S).transpose(1, 0, 2))


def kernel(q, k, v, mask, Wq, Wk, Wv, Wo, bo):
    global _BUILT
    if _BUILT is None:
        _BUILT = build_bass()
    nc = _BUILT

    f16 = np.float16
    q = np.asarray(q, np.float32)
    k = np.asarray(k, np.float32)
    v = np.asarray(v, np.float32)
    Wq = np.asarray(Wq, np.float32)
    Wk = np.asarray(Wk, np.float32)
    Wv = np.asarray(Wv, np.float32)
    Wo = np.asarray(Wo, np.float32)
    bo = np.asarray(bo, np.float32)
    mask = np.asarray(mask)

    cosT, sinT = _cos_sin_tables()
    ones64 = np.ones((1, 64), f16)
    perm = _rope_perm_cols()
    qTb = [_tile_xT(q[b].T.astype(f16)) for b in range(2)]
    kTb = [_tile_xT(k[b].T.astype(f16)) for b in range(2)]
    vTb = [_tile_vT(v[b].T.astype(f16)) for b in range(2)]
    mknegb = [_tile_mask(
        (MASK_NEG * (1.0 - mask[b, 0].T.astype(np.float32))).astype(
            ml_dtypes.float8_e5m2)) for b in range(2)]

    in_maps = []
    for c in range(N_CORES):
        b = c // 4
        head_base = (c % 4) * 4
        cols = slice(head_base * 64, head_base * 64 + 256)
        w4 = Wo[cols, :].reshape(4, 64, DIM)
        wo2 = np.stack([np.concatenate([w4[2 * hp], w4[2 * hp + 1]], axis=0)
                        for hp in range(2)], axis=1)        # [128, 2, DIM]
        in_maps.append({
            "qT": qTb[b], "kT": kTb[b], "vT": vTb[b],
            "wq": _tile_w(Wq[:, cols][:, perm].astype(f16)),
            "wk": _tile_w(Wk[:, cols][:, perm].astype(f16)),
            "wv": _tile_w(Wv[:, cols].astype(f16)),
            "wo2": np.ascontiguousarray(wo2.astype(f16)),
            "cosT": cosT, "sinT": sinT,
            "mkneg": mknegb[b], "ones64": ones64,
        })

    kernel._last_in_maps = in_maps
    res = run_bass_kernel_spmd(nc, in_maps, core_ids=list(range(N_CORES)))
    global _last_res
    _last_res = res.results
    out = np.zeros((2, S, DIM), np.float32)
    for c in range(N_CORES):
        out[c // 4] += res.results[c]["out_part"].astype(np.float32)
    out += bo[None, None, :]
    return out


# revision 19
# speedup vs baseline: 2.0034x; 1.0790x over previous
"""Multi-head attention (RoPE + mask + softmax) Trainium2 Bass kernel, v4.

Sharding: 8 cores = 2 batches x 4 head-groups. Core c handles batch c//4,
local heads 4*(c%4) .. +4 (tensor-parallel on heads; Wq/Wk/Wv column-sharded,
Wo row-sharded; per-core partial outputs summed on host).

v4: all-fp16 datapath. Scores row-tiled by head pair (K=64 halves run
concurrently). Masking is multiplicative on DVE for both heads (mask decoded
from fp8e5 -16384*(1-mask) to fp16 {0,1} on the otherwise-idle ScalarE
during phase 1). exp on ScalarE at N=1024 per (hp, m) with PSUM
double-buffering; attention m-loop software-pipelined one iteration ahead,
including across (qb, hp) block boundaries so ScalarE never gaps.
Denominator reciprocal (DRAM bounce) and normalization are emitted into
fixed slots of the NEXT q-block's m-loop so no engine stream blocks on the
bounce latency; out-projection uses K=128 head-pair matmuls (heads stacked
on 128 partitions; the upper half is lane-shifted via a SBUF->SBUF gpsimd
DMA after normalization).
"""
import sys
sys.path.insert(0, '/opt/trn_rl_repo')
import math
import numpy as np
import ml_dtypes

import concourse.bass as bass
import concourse.mybir as mybir
import concourse.tile as tile
from concourse import bacc
from concourse.bass_utils import run_bass_kernel_spmd

F32 = mybir.dt.float32
F16 = mybir.dt.float16
F8E5 = mybir.dt.float8e5

S = 2048
DIM = 1024
HEAD_DIM = 64
N_CORES = 8
KC = DIM // 128          # 8 contraction chunks for projections
MT = S // 128            # 16 k-chunks in attention
QB = 512                 # q-block width
NQB = S // QB            # 4
ROPE_THETA = 10000.0
MASK_NEG = -16384.0      # exactly representable in fp8e5

SWAP16 = [i ^ 16 for i in range(32)]   # stream_shuffle mask: lane p <-> p^16

_BUILT = None
DEBUG_DUMPS = False


def build_bass():
    nc = bacc.Bacc("TRN2", target_bir_lowering=False, debug=False)

    qT = nc.dram_tensor("qT", [4, 128, KC, 512], F16, kind="ExternalInput").ap()
    kT = nc.dram_tensor("kT", [4, 128, KC, 512], F16, kind="ExternalInput").ap()
    vT = nc.dram_tensor("vT", [MT, 128, KC, 128], F16, kind="ExternalInput").ap()
    wq = nc.dram_tensor("wq", [128, KC, 256], F16, kind="ExternalInput").ap()
    wk = nc.dram_tensor("wk", [128, KC, 256], F16, kind="ExternalInput").ap()
    wv = nc.dram_tensor("wv", [128, KC, 256], F16, kind="ExternalInput").ap()
    wo2 = nc.dram_tensor("wo2", [128, 2, DIM], F16, kind="ExternalInput").ap()
    cosT = nc.dram_tensor("cosT", [128, S], F16, kind="ExternalInput").ap()
    sinT = nc.dram_tensor("sinT", [128, S], F16, kind="ExternalInput").ap()
    mkneg = nc.dram_tensor("mkneg", [128, MT, S], F8E5, kind="ExternalInput").ap()
    ones64 = nc.dram_tensor("ones64", [1, 64], F16, kind="ExternalInput").ap()
    out_part = nc.dram_tensor("out_part", [S, DIM], F16, kind="ExternalOutput").ap()
    if DEBUG_DUMPS:
        dbg_qhT = nc.dram_tensor("dbg_qhT", [128, 2, S], F16,
                                 kind="ExternalOutput").ap()
        dbg_khT = nc.dram_tensor("dbg_khT", [128, 2, S], F16,
                                 kind="ExternalOutput").ap()
        dbg_vh = nc.dram_tensor("dbg_vh", [128, MT, 4 * 65], F16,
                                kind="ExternalOutput").ap()
        dbg_at = nc.dram_tensor("dbg_at", [8, 128, 1024], F16,
                                kind="ExternalOutput").ap()
        dbg_outT2 = nc.dram_tensor("dbg_outT2", [128, 2, S], F16,
                                   kind="ExternalOutput").ap()

    with tile.TileContext(nc) as tc:
        with tc.tile_pool(name="persist", bufs=1) as persist, \
             tc.tile_pool(name="dram", bufs=1, space="DRAM") as dram, \
             tc.tile_pool(name="ps_sc", bufs=2, space="PSUM") as ps_sc, \
             tc.tile_pool(name="ps_avp", bufs=1, space="PSUM") as ps_avp, \
             tc.tile_pool(name="ps_misc", bufs=2, space="PSUM") as ps_misc:

            qhT = persist.tile([128, 2, S], F16)     # [dim-part, hp, s]
            khT = persist.tile([128, 2, S], F16)
            vh = persist.tile([128, MT, 4 * 65], F16)
            # heads stacked for K=128 out-proj: partitions 0-63 head 2hp,
            # 64-127 head 2hp+1; slot dim = hp
            outT2 = persist.tile([128, 2, S], F16)
            wo_sb = persist.tile([128, 2, DIM], F16)
            ones_sb = persist.tile([1, 64], F16)
            cos_sb = persist.tile([128, S], F16)
            sin_sb = persist.tile([128, S], F16)
            mkneg_sb = persist.tile([128, MT, S], F8E5)
            mkmult_sb = persist.tile([128, MT, S], F16)
            dscr = dram.tile([2 * NQB, 2 * QB], F32)
            dscr2 = dram.tile([2 * NQB, 2 * QB], F16)

            nc.vector.memset(
                vh.rearrange("p m (h x) -> p m h x", x=65)[:, :, :, 64:65], 1.0)

            # ---------------- Phase 1: q/k projections + RoPE ----------------
            with tc.tile_pool(name="proj", bufs=1) as projp, \
                 tc.tile_pool(name="xts", bufs=3) as xts, \
                 tc.tile_pool(name="rope", bufs=3) as rope:
                wq_sb = projp.tile([128, KC, 256], F16)
                wk_sb = projp.tile([128, KC, 256], F16)
                wv_sb = projp.tile([128, KC, 256], F16)
                nc.sync.dma_start(out=wq_sb, in_=wq)
                nc.sync.dma_start(out=wk_sb, in_=wk)

                first = True
                for xdram, w_sb, dstT in ((qT, wq_sb, qhT), (kT, wk_sb, khT)):
                    for sblk in range(4):
                        x_sb = xts.tile([128, KC, 512], F16, tag="xts")
                        nc.sync.dma_start(out=x_sb, in_=xdram[sblk])
                        if first:
                            nc.sync.dma_start(out=cos_sb, in_=cosT)
                            nc.sync.dma_start(out=sin_sb, in_=sinT)
                            nc.sync.dma_start(out=wv_sb, in_=wv)
                            first = False
                        ss = slice(sblk * 512, (sblk + 1) * 512)
                        for hp in range(2):
                            psum = ps_sc.tile([128, 1024], F32, tag="sc")
                            for kc in range(KC):
                                nc.tensor.matmul(
                                    psum[:, 0:512],
                                    lhsT=w_sb[:, kc, hp * 128:(hp + 1) * 128],
                                    rhs=x_sb[:, kc, :],
                                    start=(kc == 0), stop=(kc == KC - 1))
                            qraw = rope.tile([128, 512], F16, tag="qraw")
                            nc.scalar.copy(qraw, psum[:, 0:512])
                            t = rope.tile([128, 512], F16, tag="t")
                            u = rope.tile([128, 512], F16, tag="u")
                            nc.vector.tensor_mul(t, qraw, cos_sb[:, ss])
                            nc.vector.tensor_mul(u, qraw, sin_sb[:, ss])
                            us = rope.tile([128, 512], F16, tag="us")
                            nc.vector.stream_shuffle(us, u, SWAP16)
                            nc.vector.tensor_add(dstT[:, hp, ss], t, us)

                # ---------------- Phase 2: v projection + mask decode -------
                # v chunks first; mask chunks trail from sc>=4 so v-proj is
                # never DMA-starved; mask decode on DVE (idle here), two
                # m-rows per op: 1 + mkneg/16384 -> {0, 1} fp16
                for sc in range(MT):
                    v_sb = xts.tile([128, KC, 128], F16, tag="xts")
                    nc.sync.dma_start(out=v_sb, in_=vT[sc])
                    if 4 <= sc < 12:
                        g = sc - 4
                        nc.sync.dma_start(out=mkneg_sb[:, g * 2:g * 2 + 2, :],
                                          in_=mkneg[:, g * 2:g * 2 + 2, :])
                    psum = ps_misc.tile([128, 512], F32, tag="misc")
                    for kc in range(KC):
                        nc.tensor.matmul(
                            psum[:, 0:256], lhsT=v_sb[:, kc, :], rhs=wv_sb[:, kc, :],
                            start=(kc == 0), stop=(kc == KC - 1))
                    nc.scalar.copy(
                        vh[:, sc, :].rearrange("p (h x) -> p h x", x=65)[:, :, 0:64],
                        psum[:, 0:256].rearrange("p (h x) -> p h x", x=64))
                    if 5 <= sc < 13:
                        g = sc - 5
                        nc.vector.tensor_scalar(
                            mkmult_sb[:, g * 2:g * 2 + 2, :],
                            mkneg_sb[:, g * 2:g * 2 + 2, :],
                            1.0 / 16384.0, 1.0,
                            op0=mybir.AluOpType.mult,
                            op1=mybir.AluOpType.add)
                nc.sync.dma_start(out=wo_sb, in_=wo2)
                nc.sync.dma_start(out=ones_sb, in_=ones64)

            # ---------------- Phase 3: attention ----------------
            with tc.tile_pool(name="at", bufs=3) as atp, \
                 tc.tile_pool(name="atm", bufs=3) as atmp, \
                 tc.tile_pool(name="stg", bufs=2) as stgp, \
                 tc.tile_pool(name="dn", bufs=4) as dnp, \
                 tc.tile_pool(name="tmp", bufs=4) as tmpp, \
                 tc.tile_pool(name="co", bufs=3) as cop:

                tmps = {}

                def emit_scores(qb, hp, m):
                    qs = slice(qb * QB, (qb + 1) * QB)
                    sct = ps_sc.tile([128, 1024], F32, tag="sc")
                    nc.tensor.matmul(
                        sct[:, 0:512],
                        lhsT=khT[0:64, hp, m * 128:(m + 1) * 128],
                        rhs=qhT[0:64, hp, qs],
                        start=True, stop=True, tile_position=(0, 0))
                    nc.tensor.matmul(
                        sct[:, 512:1024],
                        lhsT=khT[64:128, hp, m * 128:(m + 1) * 128],
                        rhs=qhT[64:128, hp, qs],
                        start=True, stop=True, tile_position=(64, 0))
                    return sct

                def outproj_chunk(qbo, j):
                    # out-projection chunk j (of 8) for q-block qbo; K=128
                    sci, nb = qbo * 4 + j // 2, j % 2
                    n5 = slice(nb * 512, (nb + 1) * 512)
                    wps = ps_misc.tile([128, 512], F32, tag="misc")
                    for sl in range(2):
                        nc.tensor.matmul(
                            wps,
                            lhsT=outT2[:, sl, sci * 128:(sci + 1) * 128],
                            rhs=wo_sb[:, sl, n5],
                            start=(sl == 0), stop=(sl == 1))
                    co = cop.tile([128, 512], F16, tag="co")
                    nc.vector.tensor_copy(co, wps)
                    nc.sync.dma_start(
                        out=out_part[sci * 128:(sci + 1) * 128, n5], in_=co)

                state = {}

                def finish_blk(bp, it):
                    """Deferred denominator/normalize for block bp=(qbp, hpp),
                    scheduled at slot `it` (m index) of the FOLLOWING block.
                    Out-projection for q-block qbp is scheduled separately."""
                    qbp, hpp = bp
                    qsp = slice(qbp * QB, (qbp + 1) * QB)
                    row = qbp * 2 + hpp
                    if it == 2:
                        rin = dnp.tile([128, 1, 8], F32, tag="rin")
                        nc.sync.dma_start(
                            out=rin,
                            in_=dscr[row:row + 1].rearrange(
                                "u (p f) -> p u f", p=128))
                        state[('rin', bp)] = rin
                    elif it == 4:
                        r32 = dnp.tile([128, 1, 8], F32, tag="r32")
                        scr = dnp.tile([128, 1, 8], F32, tag="scr")
                        nc.vector.reciprocal_approx_accurate(
                            r32, state.pop(('rin', bp)), scr)
                        rr = dnp.tile([128, 1, 8], F16, tag="rr")
                        nc.vector.tensor_copy(rr, r32)
                        nc.sync.dma_start(
                            out=dscr2[row:row + 1].rearrange(
                                "u (p f) -> p u f", p=128),
                            in_=rr)
                    elif it == 6:
                        for h2 in range(2):
                            rdn = dnp.tile([1, 512], F16, tag="rdn")
                            nc.sync.dma_start(
                                out=rdn,
                                in_=dscr2[row:row + 1,
                                          h2 * 512:(h2 + 1) * 512])
                            state[('rdn', bp, h2)] = rdn
                    elif it in (8, 9):
                        h2 = it - 8
                        pbc = ps_misc.tile([128, 512], F32, tag="misc")
                        nc.tensor.matmul(pbc[0:64, :], lhsT=ones_sb,
                                         rhs=state.pop(('rdn', bp, h2)),
                                         start=True, stop=True)
                        pbcb = dnp.tile([64, 512], F16, tag="pbcb")
                        nc.vector.tensor_copy(pbcb, pbc[0:64, :])
                        if h2 == 0:
                            nc.vector.tensor_mul(outT2[0:64, hpp, qsp],
                                                 outT2[0:64, hpp, qsp], pbcb)
                        else:
                            tmpt = tmps.pop((qbp, hpp))
                            nc.vector.tensor_mul(tmpt, tmpt, pbcb)
                            # lane-shift to partitions 64-127 for K=128 outproj
                            nc.gpsimd.dma_start(
                                out=outT2[64:128, hpp, qsp], in_=tmpt)

                blocks = [(qb, hp) for qb in range(NQB) for hp in range(2)]
                NIT = len(blocks) * MT
                avps = {}

                def blk_of(git):
                    return blocks[git // MT], git % MT

                def emit_attnv(git):
                    (qb, hp), m = blk_of(git)
                    if m == 0:
                        avps[git // MT] = ps_avp.tile(
                            [128, 1024], F32, tag="avp",
                            name=f"avp{git // MT}")
                    avp = avps[git // MT]
                    atm = state.pop(('atm', git))
                    h0, h1 = 2 * hp, 2 * hp + 1
                    nc.tensor.matmul(
                        avp[0:65, 0:512],
                        lhsT=vh[:, m, h0 * 65:(h0 + 1) * 65],
                        rhs=atm[:, 0:512],
                        start=(m == 0), stop=(m == MT - 1))
                    nc.tensor.matmul(
                        avp[0:65, 512:1024],
                        lhsT=vh[:, m, h1 * 65:(h1 + 1) * 65],
                        rhs=atm[:, 512:1024],
                        start=(m == 0), stop=(m == MT - 1))
                    if m == MT - 1:
                        # evict: h2=0 rows to outT2 lower half, h2=1 to a
                        # staging tile (normalized there, then lane-shifted),
                        # denominator row to DRAM via stg
                        qs = slice(qb * QB, (qb + 1) * QB)
                        avp = avps.pop(git // MT)
                        nc.vector.tensor_copy(outT2[0:64, hp, qs],
                                              avp[0:64, 0:512])
                        tmpt = tmpp.tile([64, 512], F16, tag="tmp")
                        nc.vector.tensor_copy(tmpt, avp[0:64, 512:1024])
                        tmps[(qb, hp)] = tmpt
                        stg = stgp.tile([128, 1024], F32, tag="stg")
                        nc.vector.tensor_copy(stg[64:65, :], avp[64:65, :])
                        nc.sync.dma_start(out=dscr[qb * 2 + hp, :],
                                          in_=stg[64:65, :])

                state[('sct', 0)] = emit_scores(0, 0, 0)
                for git in range(NIT + 1):
                    if git < NIT:
                        (qb, hp), m = blk_of(git)
                        at = atp.tile([128, 1024], F16, tag="at")
                        nc.scalar.activation(
                            at, state.pop(('sct', git)),
                            mybir.ActivationFunctionType.Exp,
                            scale=1.0 / math.sqrt(HEAD_DIM))
                        if DEBUG_DUMPS and git < 8:
                            nc.sync.dma_start(out=dbg_at[git], in_=at)
                        if git + 1 < NIT:
                            (nqb, nhp), nm = blk_of(git + 1)
                            state[('sct', git + 1)] = emit_scores(nqb, nhp, nm)
                        # multiplicative mask, both heads, on DVE
                        qs = slice(qb * QB, (qb + 1) * QB)
                        atm = atmp.tile([128, 1024], F16, tag="atm")
                        nc.vector.tensor_mul(
                            atm[:, 0:512], at[:, 0:512], mkmult_sb[:, m, qs])
                        nc.vector.tensor_mul(
                            atm[:, 512:1024], at[:, 512:1024],
                            mkmult_sb[:, m, qs])
                        state[('atm', git)] = atm
                    # attn@V one iteration behind (extra skew keeps PE fed
                    # across block boundaries while DVE digests evicts)
                    if git >= 1:
                        emit_attnv(git - 1)
                    # deferred denominator/normalize/out-proj slots
                    if git < NIT:
                        bi, m = git // MT, git % MT
                        if bi >= 1:
                            finish_blk(blocks[bi - 1], m)
                        qbc, hpc = blocks[bi]
                        if qbc >= 1:
                            if hpc == 0 and m in (11, 13, 15):
                                outproj_chunk(qbc - 1, (11, 13, 15).index(m))
                            elif hpc == 1 and m in (1, 3, 5, 7, 9):
                                outproj_chunk(
                                    qbc - 1, 3 + (1, 3, 5, 7, 9).index(m))
                # tail: last block's denominator chain, then out-projection
                for it in (2, 4, 6, 8, 9):
                    finish_blk(blocks[-1], it)
                for j in range(8):
                    outproj_chunk(NQB - 1, j)
                if DEBUG_DUMPS:
                    nc.sync.dma_start(out=dbg_qhT, in_=qhT)
                    nc.sync.dma_start(out=dbg_khT, in_=khT)
                    nc.sync.dma_start(out=dbg_vh, in_=vh)
                    nc.sync.dma_start(out=dbg_outT2, in_=outT2)

    nc.compile()
    return nc


def _rope_perm_cols():
    """Column permutation of the 256-wide W slice for one core's 4 heads.

    Chunk hp (0,1) holds local heads 2hp, 2hp+1 as partition halves.
    Within a head's 64 rows: [even 0-30 | odd 1-31 | even 32-62 | odd 33-63]
    (16 each) so the RoPE partner is at lane^16.
    """
    cols = []
    for c in range(2):
        for j2 in range(2):          # which head within the chunk
            head = 2 * c + j2
            for half in range(2):    # dims 0-31, dims 32-63
                for par in range(2):     # even (x1), odd (x2)
                    for i in range(16):
                        cols.append(head * 64 + half * 32 + 2 * i + par)
    return np.array(cols)


def _cos_sin_tables():
    inv_freq = 1.0 / (ROPE_THETA ** (np.arange(0, HEAD_DIM, 2, dtype=np.float64)
                                     / HEAD_DIM))          # [32]
    ang = np.arange(S, dtype=np.float64)[None, :] * inv_freq[:, None]  # [32, S]
    cos32 = np.cos(ang)
    sin32 = np.sin(ang)
    # row blocks of 16 per 64-row head: [c0:16, c0:16, c16:32, c16:32]
    cos64 = np.concatenate([cos32[0:16], cos32[0:16],
                            cos32[16:32], cos32[16:32]], axis=0)
    sin64 = np.concatenate([sin32[0:16], -sin32[0:16],
                            sin32[16:32], -sin32[16:32]], axis=0)
    cosT = np.tile(cos64, (2, 1)).astype(np.float16)        # [128, S]
    sinT = np.tile(sin64, (2, 1)).astype(np.float16)
    return np.ascontiguousarray(cosT), np.ascontiguousarray(sinT)


def _tile_xT(xT):
    # [1024, 2048] -> [4 sblk, 128 part, 8 kc, 512]
    return np.ascontiguousarray(
        xT.reshape(KC, 128, 4, 512).transpose(2, 1, 0, 3))


def _tile_vT(vT):
    # [1024, 2048] -> [16 sc, 128 part, 8 kc, 128]
    return np.ascontiguousarray(
        vT.reshape(KC, 128, MT, 128).transpose(2, 1, 0, 3))


def _tile_w(w):
    # [1024, 256] -> [128, 8, 256]
    return np.ascontiguousarray(w.reshape(KC, 128, 256).transpose(1, 0, 2))


def _tile_mask(maskT):
    # [2048 k, 2048 q] -> [128, 16 m, 2048]
    return np.ascontiguousarray(
        maskT.reshape(MT, 128, S).transpose(1, 0, 2))


def kernel(q, k, v, mask, Wq, Wk, Wv, Wo, bo):
    global _BUILT
    if _BUILT is None:
        _BUILT = build_bass()
    nc = _BUILT

    f16 = np.float16
    q = np.asarray(q, np.float32)
    k = np.asarray(k, np.float32)
    v = np.asarray(v, np.float32)
    Wq = np.asarray(Wq, np.float32)
    Wk = np.asarray(Wk, np.float32)
    Wv = np.asarray(Wv, np.float32)
    Wo = np.asarray(Wo, np.float32)
    bo = np.asarray(bo, np.float32)
    mask = np.asarray(mask)

    cosT, sinT = _cos_sin_tables()
    ones64 = np.ones((1, 64), f16)
    perm = _rope_perm_cols()
    qTb = [_tile_xT(q[b].T.astype(f16)) for b in range(2)]
    kTb = [_tile_xT(k[b].T.astype(f16)) for b in range(2)]
    vTb = [_tile_vT(v[b].T.astype(f16)) for b in range(2)]
    mknegb = [_tile_mask(
        (MASK_NEG * (1.0 - mask[b, 0].T.astype(np.float32))).astype(
            ml_dtypes.float8_e5m2)) for b in range(2)]

    in_maps = []
    for c in range(N_CORES):
        b = c // 4
        head_base = (c % 4) * 4
        cols = slice(head_base * 64, head_base * 64 + 256)
        w4 = Wo[cols, :].reshape(4, 64, DIM)
        wo2 = np.stack([np.concatenate([w4[2 * hp], w4[2 * hp + 1]], axis=0)
                        for hp in range(2)], axis=1)        # [128, 2, DIM]
        in_maps.append({
            "qT": qTb[b], "kT": kTb[b], "vT": vTb[b],
            "wq": _tile_w(Wq[:, cols][:, perm].astype(f16)),
            "wk": _tile_w(Wk[:, cols][:, perm].astype(f16)),
            "wv": _tile_w(Wv[:, cols].astype(f16)),
            "wo2": np.ascontiguousarray(wo2.astype(f16)),
            "cosT": cosT, "sinT": sinT,
            "mkneg": mknegb[b], "ones64": ones64,
        })

    kernel._last_in_maps = in_maps
    res = run_bass_kernel_spmd(nc, in_maps, core_ids=list(range(N_CORES)))
    global _last_res
    _last_res = res.results
    out = np.zeros((2, S, DIM), np.float32)
    for c in range(N_CORES):
        out[c // 4] += res.results[c]["out_part"].astype(np.float32)
    out += bo[None, None, :]
    return out
